# revision 34
# baseline (speedup 1.0000x reference)
"""Trainium2 Bass kernel for nn_AgentRNN: dual GRU scan + subtask/Q heads.

Sharding (8 cores, data-parallel, no collectives):
  - GRU2/Q path ("branch 2"): core c owns agent n=c -> rows [128c, 128(c+1))
    of the NB=1024 flat batch (nb-major).
  - GRU1/logits path ("branch 1"): core c owns batch columns b in
    [16c, 16(c+1)) across all 8 agents, with rows ordered b-major
    (local row j <-> (b = 16c + j//8, n = j%8)).
    With this choice, prob_flat row i == q row i live on the same core
    (prob_flat maps i -> (b=i//8, n=i%8), i.e. j == i - 128c), so the
    q_vals one-hot gather needs no cross-core traffic.

Precision:
  - Branch 1 feeds an argmax over logits whose top-2 gaps go down to ~3e-9,
    so its whole chain (embed, gi, gh) runs as 3-term bf16 split matmuls
    (x ~ xh+xl, W ~ Wh+Wl; terms xh@Wh + xl@Wh + xh@Wl), giving ~2^-17
    effective input precision at 3 bf16 matmul passes (vs 4 for true fp32).
    The logits head itself (y1 @ W_le) runs in true fp32.
  - Branch 2 only feeds Q *values* (no selection), so single-term bf16.

Device layout per core (R=128 rows per branch):
  - batch layout: partitions = local rows (128), free = H
  - T layout:     tiles (128, 512) where column block j holds
                  h[:, 128j:128(j+1)].T  (matmul lhsT operands)
"""
import sys
import os

sys.path.insert(0, "/opt/trn_rl_repo")

import numpy as np
import ml_dtypes

BF = ml_dtypes.bfloat16

T, N, B, D = 64, 8, 128, 128
H, S, K, A = 512, 64, 16, 16
NB = N * B
R = 128          # rows per core per branch
NCORES = 8
HC = H // 128    # 4 h-chunks

_CACHE = {}


def _build_program(cfg):
    """cfg: (bi1_nz, bhn1_nz, bi2_nz, bhn2_nz, ble_nz, bemb1_nz, t_steps)"""
    import concourse.bacc as bacc
    import concourse.tile as tile
    from concourse import mybir

    bi_nz = {1: cfg[0], 2: cfg[2]}
    bhn_nz = {1: cfg[1], 2: cfg[3]}
    ble_nz = cfg[4]
    bemb1_nz = cfg[5]
    h0z = cfg[6]
    TS = cfg[7]

    f32 = mybir.dt.float32
    bf16 = mybir.dt.bfloat16
    AF = mybir.ActivationFunctionType
    ALU = mybir.AluOpType
    AX = mybir.AxisListType

    nc = bacc.Bacc("TRN2", num_devices=NCORES)

    din = {}

    def inp(name, shape, dt):
        din[name] = nc.dram_tensor(name, list(shape), dt, kind="ExternalInput")
        return din[name]

    # branch 1 (split operands), branch 2 (single bf16)
    inp("obsT1h", (TS, D, R), bf16)
    inp("obsT1l", (TS, D, R), bf16)
    inp("obsT2", (TS, D, R), bf16)
    for sfx in ("h", "l"):
        inp(f"wi1{sfx}", (128, HC * 3 * H), bf16)
        inp(f"wh1{sfx}", (128, HC * 3 * H), bf16)
        inp(f"wembed{sfx}", (128, H), bf16)
    inp("wi2", (128, HC * 3 * H), bf16)
    inp("wh2", (128, HC * 3 * H), bf16)
    inp("wpol", (128, H), bf16)
    for br in (1, 2):
        inp(f"ndt{br}", (128, TS), f32)
        inp(f"ndb{br}", (TS, R), bf16)
        inp(f"h{br}b0", (R, H), f32)
    inp("h1t0h", (128, H), bf16)
    inp("h1t0l", (128, H), bf16)
    inp("h2t0", (128, H), bf16)
    inp("bemb", (128, HC), f32)
    inp("bpol", (128, HC), f32)
    inp("w1f", (128, HC * K * A), bf16)
    inp("wle", (128, HC * K), f32)
    inp("b1f", (1, K * A), bf16)
    inp("onesb", (1, R), bf16)
    inp("onesf", (1, R), f32)
    inp("ident", (128, 128), f32)
    if bi_nz[1]:
        inp("bi1h", (1, 3 * H), bf16)
        inp("bi1l", (1, 3 * H), bf16)
    if bhn_nz[1]:
        inp("bhn1h", (1, H), bf16)
        inp("bhn1l", (1, H), bf16)
    if bi_nz[2]:
        inp("bi2", (1, 3 * H), bf16)
    if bhn_nz[2]:
        inp("bhn2", (1, H), bf16)
    if ble_nz:
        inp("ble", (1, K), f32)

    out_h1 = nc.dram_tensor("h1f", [R, H], f32, kind="ExternalOutput")
    out_h2 = nc.dram_tensor("h2f", [R, H], f32, kind="ExternalOutput")
    out_lg = nc.dram_tensor("lg1", [R, TS * K], f32, kind="ExternalOutput")
    out_qv = nc.dram_tensor("qv2", [R, TS * A], f32, kind="ExternalOutput")

    with tile.TileContext(nc) as tc:
        _emit(nc, tc, din, out_h1, out_h2, out_lg, out_qv,
              bi_nz, bhn_nz, ble_nz, bemb1_nz, h0z, TS, f32, bf16, AF, ALU, AX)
    nc.compile()
    return nc


def _dedup_ldweights(nc):
    """Remove InstLdweights that reload the exact weights already resident:
    consecutive-on-the-PE-stream LDWs with an identical stationary AP (only
    non-transpose matmuls in between) are redundant -- the PE array still
    holds the data.  Waits from a removed LDW migrate to the next matmul."""
    from concourse import mybir

    removed = kept = 0
    for fn in nc.m.functions:
        for blk in fn.blocks:
            new_insts = []
            last_sig = None
            pending_waits = []
            for inst in blk.instructions:
                tn = type(inst).__name__
                if tn == "InstLdweights":
                    ap = inst.ins[-1]
                    sig = (ap.memref, ap.offset,
                           tuple(tuple(p) for p in ap.ap), str(ap.dtype),
                           inst.perf_mode, inst.is_transpose,
                           inst.tile_position)
                    if sig == last_sig:
                        removed += 1
                        si = inst.sync_info
                        if si is not None:
                            pending_waits.extend(si.on_wait)
                            assert not si.on_update, \
                                "removed LDW carries sem updates"
                        continue
                    last_sig = sig
                    kept += 1
                elif tn == "InstMatmult":
                    if pending_waits:
                        si = inst.sync_info
                        if si is None:
                            inst.sync_info = mybir.SyncInfo(
                                on_wait=list(pending_waits), on_update=[])
                        else:
                            have = {(w.id, w.wait_value) for w in si.on_wait}
                            for w in pending_waits:
                                if (w.id, w.wait_value) not in have:
                                    si.on_wait.append(w)
                        pending_waits = []
                new_insts.append(inst)
            assert not pending_waits, "dangling waits from removed LDW"
            blk.instructions[:] = new_insts
    print(f"[kernel] ldweights dedup: removed {removed}, kept {kept}")
    return nc


def _emit(nc, tc, din, out_h1, out_h2, out_lg, out_qv,
          bi_nz, bhn_nz, ble_nz, bemb1_nz, h0z, TS, f32, bf16, AF, ALU, AX):
    from contextlib import ExitStack

    ctx = ExitStack()
    with ctx:
        const = ctx.enter_context(tc.tile_pool(name="const", bufs=1))
        obs_p = ctx.enter_context(tc.tile_pool(name="obs", bufs=3))
        ae_p = ctx.enter_context(tc.tile_pool(name="aeT", bufs=4))
        st_p = ctx.enter_context(tc.tile_pool(name="state", bufs=2))
        tmp_p = ctx.enter_context(tc.tile_pool(name="tmp", bufs=8))
        outp = ctx.enter_context(tc.tile_pool(name="outs", bufs=1))
        ps_g = ctx.enter_context(tc.tile_pool(name="psg", bufs=4, space="PSUM"))
        ps_tr = ctx.enter_context(tc.tile_pool(name="pstr", bufs=2, space="PSUM"))
        ps_m = ctx.enter_context(tc.tile_pool(name="psm", bufs=2, space="PSUM"))

        def ctile(name, dt):
            t_ = const.tile(list(din[name].shape), dt, tag=name, name=name)
            nc.sync.dma_start(t_[:], din[name].ap())
            return t_

        # weights: per branch, dict suffix -> (wi, wh)
        wi = {1: {"h": ctile("wi1h", bf16), "l": ctile("wi1l", bf16)},
              2: {"h": ctile("wi2", bf16)}}
        wh = {1: {"h": ctile("wh1h", bf16), "l": ctile("wh1l", bf16)},
              2: {"h": ctile("wh2", bf16)}}
        wemb = {1: {"h": ctile("wembedh", bf16), "l": ctile("wembedl", bf16)},
                2: {"h": ctile("wpol", bf16)}}
        # (data_sfx, weight_sfx) term lists
        terms = {1: (("h", "h"), ("l", "h"), ("h", "l")), 2: (("h", "h"),)}

        bemb = {1: ctile("bemb", f32), 2: ctile("bpol", f32)}
        ndt = {1: ctile("ndt1", f32), 2: ctile("ndt2", f32)}
        ndb_t = {}  # (gidx, br) -> (1, ng*R) sbuf tile, loaded with the group
        w1f = ctile("w1f", bf16)
        wle = ctile("wle", f32)
        b1f = ctile("b1f", bf16)
        onesb = ctile("onesb", bf16)
        onesf = ctile("onesf", f32)
        ident = ctile("ident", f32)
        bi = {}
        bhn = {}
        if bi_nz[1]:
            bi[1] = {"h": ctile("bi1h", bf16), "l": ctile("bi1l", bf16)}
        if bhn_nz[1]:
            bhn[1] = {"h": ctile("bhn1h", bf16), "l": ctile("bhn1l", bf16)}
        if bi_nz[2]:
            bi[2] = {"h": ctile("bi2", bf16)}
        if bhn_nz[2]:
            bhn[2] = {"h": ctile("bhn2", bf16)}
        ble = ctile("ble", f32) if ble_nz else None

        hB = {br: ctile(f"h{br}b0", f32) for br in (1, 2)}
        hT = {1: {"h": ctile("h1t0h", bf16), "l": ctile("h1t0l", bf16)},
              2: {"h": ctile("h2t0", bf16)}}

        oh_sb = outp.tile([128, TS * K], f32, tag="oh")
        lg_sb = outp.tile([128, TS * K], f32, tag="lg")
        qv_sb = outp.tile([128, TS * A], f32, tag="qv")

        aeT = {1: {}, 2: {}}
        grp_w = {}

        aeT_groups = {}

        def emit_embed(gidx0, br):
            t0 = 4 * gidx0
            if t0 >= TS:
                return
            ng_t = min(4, TS - t0)  # timesteps in this obs group
            obs_d = {}
            for dsfx in ("h", "l") if br == 1 else ("h",):
                nm = f"obsT{br}{dsfx}" if br == 1 else "obsT2"
                ob = obs_p.tile([128, ng_t * R], bf16,
                                tag=f"ob{br}{dsfx}", name=f"ob{br}{dsfx}")
                nc.sync.dma_start(
                    ob[:].rearrange("d (t b) -> d t b", t=ng_t),
                    din[nm].ap()[t0:t0 + ng_t].rearrange("t d b -> d t b"))
                obs_d[dsfx] = ob
            nb = obs_p.tile([1, ng_t * R], bf16, tag=f"nb{br}", name=f"nb{br}",
                            bufs=3)
            nc.sync.dma_start(
                nb[:],
                din[f"ndb{br}"].ap().rearrange("(o t) b -> o (t b)", o=1)[
                    0:1, t0 * R:(t0 + ng_t) * R])
            ndBp4 = ps_tr.tile([128, ng_t * R], f32, tag="tr", name="ndBp4")
            nc.tensor.matmul(ndBp4[:], onesb[:, :], nb[:], start=True, stop=True)
            nd4 = st_p.tile([128, ng_t * R], f32, tag=f"nd4{br}", name=f"nd4{br}",
                            bufs=3)
            nc.vector.tensor_copy(nd4[:], ndBp4[:])
            ndb_t[(gidx0, br)] = nd4
            emb_terms = {1: (("h", "h"), ("l", "h"), ("h", "l")),
                         2: (("h", "h"),)}[br]
            if br == 1:
                ae1h = ae_p.tile([128, HC * ng_t * R], bf16, tag="ae1h",
                                 name="ae1h", bufs=3)
                ae1l = ae_p.tile([128, HC * ng_t * R], bf16, tag="ae1l",
                                 name="ae1l", bufs=3)
            else:
                ae2 = ae_p.tile([128, HC * ng_t * R], bf16, tag="ae2",
                                name="ae2", bufs=3)
            for hc in range(HC):
                pa = ps_m.tile([128, ng_t * R], f32, tag="m", name="pa")
                for i, (ds, ws) in enumerate(emb_terms):
                    nc.tensor.matmul(
                        pa[:], wemb[br][ws][:, hc * 128:(hc + 1) * 128],
                        obs_d[ds][:], start=(i == 0),
                        stop=(i == len(emb_terms) - 1), skip_group_check=True)
                sl = slice(hc * ng_t * R, (hc + 1) * ng_t * R)
                if br == 2:
                    nc.scalar.activation(ae2[:, sl], pa[:], AF.Relu,
                                         bias=bemb[2][:, hc:hc + 1])
                elif bemb1_nz:
                    aef = tmp_p.tile([128, ng_t * R], f32, tag="aef",
                                     bufs=1, name="aef")
                    nc.scalar.activation(aef[:], pa[:], AF.Relu,
                                         bias=bemb[1][:, hc:hc + 1])
                    nc.vector.tensor_copy(ae1h[:, sl], aef[:])
                    nc.vector.tensor_sub(ae1l[:, sl], aef[:], ae1h[:, sl])
                else:
                    nc.scalar.activation(ae1h[:, sl], pa[:], AF.Relu)
                    # ael = relu(ps) - aeh   (one fused DVE op)
                    nc.vector.scalar_tensor_tensor(
                        ae1l[:, sl], pa[:], 0.0, ae1h[:, sl],
                        ALU.max, ALU.subtract)
            if br == 1:
                aeT_groups[(gidx0, 1)] = ({"h": ae1h, "l": ae1l}, ng_t * R)
            else:
                aeT_groups[(gidx0, 2)] = ({"h": ae2}, ng_t * R)

        for gg in (0, 1):
            for brr in (1, 2):
                emit_embed(gg, brr)
        for t in range(TS):
            if t % 4 == 0:
                for brr in (1, 2):
                    aeT[brr], grp_w[brr] = aeT_groups.pop((t // 4, brr))
            tl = t % 4

            gidx = {"r": 0, "z": 1, "inn": 2, "hn": 2}

            def mm_phase(br):
                # gi + gh accumulated per gate bank; phases ordered so r/z
                # close early; kc-major, lhsT-grouped for ldweights dedup.
                g = {gate: ps_g.tile([128, 512], f32, tag="g", name=f"g{gate}")
                     for gate in ("r", "z", "hn", "inn")}
                dws = {}
                for ds, ws in terms[br]:
                    dws.setdefault(ds, []).append(ws)

                def gi_lhs(kc, ds):
                    return aeT[br][ds][:, kc * grp_w[br] + tl * R:
                                       kc * grp_w[br] + tl * R + R]

                def gh_lhs(kc, ds):
                    return hT[br][ds][:, kc * 128:(kc + 1) * 128]

                def wslice(w, ws, kc, gate):
                    return w[ws][:, kc * 1536 + gidx[gate] * 512:
                                 kc * 1536 + gidx[gate] * 512 + 512]

                skip_gh = h0z and t == 0
                sched = []  # (bank, lhsT_ap, rhs_ap)
                for kc in range(HC):
                    for ds, wss in dws.items():
                        for ws in wss:
                            for gate in ("r", "z"):
                                sched.append((gate, gi_lhs(kc, ds),
                                              wslice(wi[br], ws, kc, gate)))
                if bi_nz[br]:
                    for ws in bi[br]:
                        for gate in ("r", "z"):
                            sched.append((gate, onesb[:, :],
                                          bi[br][ws][:, gidx[gate] * 512:
                                                     gidx[gate] * 512 + 512]))
                if not skip_gh:
                    for kc in range(HC):
                        for ds, wss in dws.items():
                            for ws in wss:
                                for gate in ("r", "z"):
                                    sched.append((gate, gh_lhs(kc, ds),
                                                  wslice(wh[br], ws, kc, gate)))
                    for kc in range(HC):
                        for ds, wss in dws.items():
                            for ws in wss:
                                sched.append(("hn", gh_lhs(kc, ds),
                                              wslice(wh[br], ws, kc, "hn")))
                if bhn_nz[br]:
                    for ws in bhn[br]:
                        sched.append(("hn", onesb[:, :], bhn[br][ws][:, :]))
                for kc in range(HC):
                    for ds, wss in dws.items():
                        for ws in wss:
                            sched.append(("inn", gi_lhs(kc, ds),
                                          wslice(wi[br], ws, kc, "inn")))
                if bi_nz[br]:
                    for ws in bi[br]:
                        sched.append(("inn", onesb[:, :],
                                      bi[br][ws][:, gidx["inn"] * 512:
                                                 gidx["inn"] * 512 + 512]))
                total = {}
                for gate, _, _ in sched:
                    total[gate] = total.get(gate, 0) + 1
                seen = {gate: 0 for gate in total}
                for gate, lhs, rhs in sched:
                    seen[gate] += 1
                    nc.tensor.matmul(g[gate][:], lhs, rhs,
                                     start=(seen[gate] == 1),
                                     stop=(seen[gate] == total[gate]),
                                     skip_group_check=True)
                return g

            def gates_tail(br, g):
                skip_gh = h0z and t == 0
                zg = tmp_p.tile([128, 512], f32, tag="tmp", bufs=7, name="zg")
                nc.scalar.activation(zg[:], g["z"][:], AF.Sigmoid)
                ngt = tmp_p.tile([128, 512], f32, tag="tmp", bufs=7, name="ngt")
                if skip_gh:
                    nc.scalar.activation(ngt[:], g["inn"][:], AF.Tanh)
                else:
                    rg = tmp_p.tile([128, 512], f32, tag="tmp", bufs=7, name="rg")
                    nc.scalar.activation(rg[:], g["r"][:], AF.Sigmoid)
                    m = tmp_p.tile([128, 512], f32, tag="tmp", bufs=7, name="m")
                    nc.vector.tensor_tensor(m[:], rg[:], g["hn"][:], ALU.mult)
                    nin = tmp_p.tile([128, 512], f32, tag="tmp", bufs=7, name="nin")
                    nc.vector.tensor_tensor(nin[:], m[:], g["inn"][:], ALU.add)
                    nc.scalar.activation(ngt[:], nin[:], AF.Tanh)
                d = tmp_p.tile([128, 512], f32, tag="tmp", bufs=7, name="d")
                nc.vector.tensor_sub(d[:], hB[br][:], ngt[:])
                p_ = tmp_p.tile([128, 512], f32, tag="tmp", bufs=7, name="p_")
                nc.vector.tensor_mul(p_[:], zg[:], d[:])
                hnew = tmp_p.tile([128, 512], f32, tag="hnew", bufs=3, name="hnew")
                nc.vector.tensor_add(hnew[:], ngt[:], p_[:])
                if t == TS - 1:
                    nc.sync.dma_start((out_h1 if br == 1 else out_h2).ap(),
                                      hnew[:])
                # masked next-state (batch layout)
                hBn = st_p.tile([128, 512], f32, tag=f"hB{br}", name=f"hB{br}")
                nc.vector.tensor_scalar_mul(hBn[:], hnew[:], ndt[br][:, t:t + 1])
                hB[br] = hBn
                # single transpose set (unmasked); T-layout mask comes from a
                # rank-1 ones x nd-row matmul broadcast across partitions
                tru = ps_tr.tile([128, 512], f32, tag="tr", name="tru")
                for j in range(4):
                    nc.tensor.transpose(tru[:, j * 128:(j + 1) * 128],
                                        hnew[:, j * 128:(j + 1) * 128], ident[:])
                ndB3 = ndb_t[(t // 4, br)][:, tl * R:(tl + 1) * R].rearrange(
                    "p (o b) -> p o b", o=1).to_broadcast((128, 4, 128))
                tru3 = tru[:].rearrange("p (j b) -> p j b", j=4)
                if br == 1:
                    t1 = tmp_p.tile([128, 512], f32, tag="tmp", bufs=7, name="t1")
                    nc.vector.tensor_tensor(
                        t1[:].rearrange("p (j b) -> p j b", j=4), tru3, ndB3,
                        ALU.mult)
                    hTh = st_p.tile([128, 512], bf16, tag="hT1h", name="hT1h")
                    nc.scalar.activation(hTh[:], t1[:], AF.Copy)
                    hTl = st_p.tile([128, 512], bf16, tag="hT1l", name="hT1l")
                    nc.vector.tensor_sub(hTl[:], t1[:], hTh[:])
                    hT[1] = {"h": hTh, "l": hTl}
                else:
                    hT2n = st_p.tile([128, 512], bf16, tag="hT2", name="hT2")
                    nc.vector.tensor_tensor(
                        hT2n[:].rearrange("p (j b) -> p j b", j=4), tru3, ndB3,
                        ALU.mult)
                    hT[2] = {"h": hT2n}

                if br == 1:
                    # logits in true fp32 + one-hot
                    yT = st_p.tile([128, 512], f32, tag="yT1", name="yT1")
                    nc.scalar.activation(yT[:], tru[:], AF.Copy)
                    lgp = ps_m.tile([128, 512], f32, tag="m", name="lgp")
                    lmms = [(yT[:, kc * 128:(kc + 1) * 128],
                             wle[:, kc * K:(kc + 1) * K]) for kc in range(HC)]
                    if ble_nz:
                        lmms.append((onesf[:, :], ble[:, :]))
                    for i, (lhs, rhs) in enumerate(lmms):
                        nc.tensor.matmul(lgp[:, 0:K], lhs, rhs, start=(i == 0),
                                         stop=(i == len(lmms) - 1))
                    nc.vector.tensor_copy(lg_sb[:, t * K:(t + 1) * K], lgp[:, 0:K])
                    mx = tmp_p.tile([128, 1], f32, tag="mx", bufs=2, name="mx")
                    nc.vector.tensor_reduce(mx[:], lgp[:, 0:K], AX.X, ALU.max)
                    nc.vector.tensor_tensor(oh_sb[:, t * K:(t + 1) * K], lgp[:, 0:K],
                                            mx[:, 0:1].to_broadcast((128, K)),
                                            ALU.is_ge)
                else:
                    # q = y2 @ w1f + b1f ; q_vals = sum_k OH * q
                    yT = st_p.tile([128, 512], bf16, tag="yT2", name="yT2")
                    nc.scalar.activation(yT[:], tru[:], AF.Copy)
                    qp = ps_m.tile([128, 512], f32, tag="m", name="qp")
                    qmms = [(yT[:, kc * 128:(kc + 1) * 128],
                             w1f[:, kc * K * A:(kc + 1) * K * A]) for kc in range(HC)]
                    qmms.append((onesb[:, :], b1f[:, :]))
                    for i, (lhs, rhs) in enumerate(qmms):
                        nc.tensor.matmul(qp[:, 0:K * A], lhs, rhs, start=(i == 0),
                                         stop=(i == len(qmms) - 1))
                    qm = tmp_p.tile([128, K * A], f32, tag="qm", bufs=2, name="qm")
                    nc.vector.tensor_tensor(
                        qm[:].rearrange("p (a k) -> p a k", k=K),
                        qp[:, 0:K * A].rearrange("p (a k) -> p a k", k=K),
                        oh_sb[:, t * K:(t + 1) * K].rearrange(
                            "p (o k) -> p o k", o=1).to_broadcast((128, A, K)),
                        ALU.mult)
                    nc.vector.tensor_reduce(qv_sb[:, t * A:(t + 1) * A],
                                            qm[:].rearrange("p (a k) -> p a k", k=K),
                                            AX.X, ALU.add)

            g1 = mm_phase(1)
            g2 = mm_phase(2)
            gates_tail(1, g1)
            if t % 4 == 0:
                emit_embed(t // 4 + 2, 1)
            elif t % 4 == 2:
                emit_embed(t // 4 + 2, 2)
            gates_tail(2, g2)

        nc.sync.dma_start(out_lg.ap(), lg_sb[:])
        nc.sync.dma_start(out_qv.ap(), qv_sb[:])


def _bf16_split(x):
    xh = x.astype(BF)
    xl = (x - xh.astype(np.float32)).astype(BF)
    return xh, xl


def _host_prep(inputs):
    f = lambda x: np.ascontiguousarray(np.asarray(x), dtype=np.float32)
    W_embed, b_embed = f(inputs["W_embed"]), f(inputs["b_embed"])
    Wi1, bi1 = f(inputs["Wi1"]), f(inputs["bi1"])
    Wh1, bhn1 = f(inputs["Wh1"]), f(inputs["bhn1"])
    W_sub, b_sub = f(inputs["W_sub"]), f(inputs["b_sub"])
    W_e1, b_e1 = f(inputs["W_e1"]), f(inputs["b_e1"])
    W_e2, b_e2 = f(inputs["W_e2"]), f(inputs["b_e2"])
    W_pol, b_pol = f(inputs["W_pol"]), f(inputs["b_pol"])
    Wi2, bi2 = f(inputs["Wi2"]), f(inputs["bi2"])
    Wh2, bhn2 = f(inputs["Wh2"]), f(inputs["bhn2"])
    W_w1, b_w1 = f(inputs["W_w1"]), f(inputs["b_w1"])
    W_b1, b_b1 = f(inputs["W_b1"]), f(inputs["b_b1"])

    e = np.tanh(np.maximum(W_e1 + b_e1, 0.0) @ W_e2 + b_e2)      # (K, S)
    W_le = W_sub @ e.T                                           # (H, K)
    b_le = b_sub @ e.T                                           # (K,)
    w1 = (e @ W_w1 + b_w1).reshape(K, H, A)                      # (K, H, A)
    b1 = e @ W_b1 + b_b1                                         # (K, A)
    # q matmul columns ordered (a, k): col a*K + k = w1[k, :, a]
    W1f = np.ascontiguousarray(w1.transpose(2, 0, 1).reshape(K * A, H).T)
    b1f = np.ascontiguousarray(b1.T.reshape(1, K * A))

    blk = lambda M: np.ascontiguousarray(
        np.concatenate([M[kc * 128:(kc + 1) * 128, :]
                        for kc in range(M.shape[0] // 128)], axis=1))
    wi1h, wi1l = _bf16_split(blk(Wi1))
    wh1h, wh1l = _bf16_split(blk(Wh1))
    wembedh, wembedl = _bf16_split(W_embed)
    prep = dict(
        wi1h=wi1h, wi1l=wi1l, wh1h=wh1h, wh1l=wh1l,
        wembedh=wembedh, wembedl=wembedl,
        wi2=blk(Wi2).astype(BF), wh2=blk(Wh2).astype(BF),
        wpol=W_pol.astype(BF),
        bemb=np.ascontiguousarray(b_embed.reshape(HC, 128).T),
        bpol=np.ascontiguousarray(b_pol.reshape(HC, 128).T),
        w1f=blk(W1f).astype(BF), wle=blk(W_le),
        b1f=b1f.astype(BF),
        onesb=np.ones((1, R), BF), onesf=np.ones((1, R), np.float32),
        ident=np.eye(128, dtype=np.float32),
        e=e,
    )
    flags = dict(
        bi1_nz=bool(np.any(bi1)), bhn1_nz=bool(np.any(bhn1)),
        bi2_nz=bool(np.any(bi2)), bhn2_nz=bool(np.any(bhn2)),
        ble_nz=bool(np.any(b_le)), bemb1_nz=bool(np.any(b_embed)),
        h0z=not (np.any(np.asarray(inputs["h1"])) or
                 np.any(np.asarray(inputs["h2"]))),
    )
    if flags["bi1_nz"]:
        prep["bi1h"], prep["bi1l"] = _bf16_split(bi1.reshape(1, 3 * H))
    if flags["bhn1_nz"]:
        prep["bhn1h"], prep["bhn1l"] = _bf16_split(bhn1.reshape(1, H))
    if flags["bi2_nz"]:
        prep["bi2"] = bi2.reshape(1, 3 * H).astype(BF)
    if flags["bhn2_nz"]:
        prep["bhn2"] = bhn2.reshape(1, H).astype(BF)
    if flags["ble_nz"]:
        prep["ble"] = b_le.reshape(1, K)
    return prep, flags


def _tlayout(h):
    """(R, H) batch-layout -> (128, H) T-layout (column block j = chunk j^T)."""
    return np.ascontiguousarray(
        h.T.reshape(HC, 128, R).transpose(1, 0, 2).reshape(128, HC * R))


def _core_inputs(inputs, prep, flags, c, t_steps):
    obs = np.asarray(inputs["obs"], dtype=np.float32)
    done = np.asarray(inputs["done"])
    nd = 1.0 - done.astype(np.float32)          # (T, NB)
    h1 = np.asarray(inputs["h1"], dtype=np.float32)
    h2 = np.asarray(inputs["h2"], dtype=np.float32)

    m = {}
    # branch 2: agent shard, rows nb-major
    rows2 = slice(c * R, (c + 1) * R)
    obs2 = np.ascontiguousarray(obs[:, c].transpose(0, 2, 1))[:t_steps]
    m["obsT2"] = obs2.astype(BF)
    nd2 = nd[:, rows2]                          # (T, R)
    ndn2 = np.vstack([nd2[1:], np.ones((1, R), np.float32)])
    m["ndt2"] = np.ascontiguousarray(ndn2.T)[:, :t_steps]
    m["ndb2"] = ndn2[:t_steps].astype(BF)
    h2m = h2[rows2] * nd2[0][:, None]
    m["h2b0"] = np.ascontiguousarray(h2m)
    m["h2t0"] = _tlayout(h2m).astype(BF)

    # branch 1: b-slice shard, rows b-major: local j = r_*8 + n, b = 16c + r_
    bsl = slice(16 * c, 16 * (c + 1))
    Tn = obs.shape[0]
    obs1 = obs[:, :, bsl, :].transpose(0, 2, 1, 3).reshape(Tn, R, D)
    obs1T = np.ascontiguousarray(obs1.transpose(0, 2, 1))[:t_steps]
    m["obsT1h"], m["obsT1l"] = _bf16_split(obs1T)
    done_v = done.reshape(Tn, N, B)
    nd1 = (1.0 - done_v[:, :, bsl].astype(np.float32)).transpose(0, 2, 1).reshape(Tn, R)
    ndn1 = np.vstack([nd1[1:], np.ones((1, R), np.float32)])
    m["ndt1"] = np.ascontiguousarray(ndn1.T)[:, :t_steps]
    m["ndb1"] = ndn1[:t_steps].astype(BF)
    h1_v = h1.reshape(N, B, H)[:, bsl].transpose(1, 0, 2).reshape(R, H)
    h1m = h1_v * nd1[0][:, None]
    m["h1b0"] = np.ascontiguousarray(h1m)
    m["h1t0h"], m["h1t0l"] = _bf16_split(_tlayout(h1m))

    for k in ("wi1h", "wi1l", "wh1h", "wh1l", "wembedh", "wembedl",
              "wi2", "wh2", "wpol", "bemb", "bpol", "w1f", "wle", "b1f",
              "onesb", "onesf", "ident"):
        m[k] = prep[k]
    for k, fl in (("bi1h", "bi1_nz"), ("bi1l", "bi1_nz"), ("bi2", "bi2_nz"),
                  ("bhn1h", "bhn1_nz"), ("bhn1l", "bhn1_nz"),
                  ("bhn2", "bhn2_nz"), ("ble", "ble_nz")):
        if flags[fl]:
            m[k] = prep[k]
    return m


def get_program(flags, t_steps=T):
    key = (flags["bi1_nz"], flags["bhn1_nz"], flags["bi2_nz"], flags["bhn2_nz"],
           flags["ble_nz"], flags["bemb1_nz"], flags["h0z"], t_steps)
    if key not in _CACHE:
        _CACHE[key] = _build_program(key)
    return _CACHE[key]


def assemble(results, prep, t_steps=T):
    """Gather per-core results into the full output pytree."""
    h1f = np.empty((NB, H), np.float32)
    h2f = np.empty((NB, H), np.float32)
    qv = np.empty((t_steps, NB, A), np.float32)
    lg = np.empty((t_steps, B, N, K), np.float32)
    for c in range(NCORES):
        r = results[c]
        h2f[c * R:(c + 1) * R] = r["h2f"]
        qv[:, c * R:(c + 1) * R] = r["qv2"].reshape(R, t_steps, A).transpose(1, 0, 2)
        # branch1 rows: j = r_*8 + n  -> (b = 16c + r_, n)
        l_ = r["lg1"].reshape(16, 8, t_steps, K)      # (r_, n, t, k)
        lg[:, 16 * c:16 * (c + 1)] = l_.transpose(2, 0, 1, 3)
        h1_ = r["h1f"].reshape(16, 8, H)              # (r_, n, H)
        h1f.reshape(N, B, H)[:, 16 * c:16 * (c + 1)] = h1_.transpose(1, 0, 2)
    se = np.broadcast_to(prep["e"], (t_steps, B, K, S)).copy()
    return ((h1f, h2f), qv, lg, se)


LAST_EXEC_NS = None


def kernel(_trace=False, **inputs):
    global LAST_EXEC_NS
    from concourse.bass_utils import run_bass_kernel_spmd

    prep, flags = _host_prep(inputs)
    nc = get_program(flags)
    in_maps = [_core_inputs(inputs, prep, flags, c, T) for c in range(NCORES)]
    core_ids = list(range(NCORES))
    if _trace:
        try:
            res = run_bass_kernel_spmd(nc, in_maps, core_ids=core_ids, trace=True)
        except Exception as e:  # trace infra (NTFF hook / upload) can fail
            print(f"[kernel] trace run failed ({e!r}); rerunning untraced")
            res = run_bass_kernel_spmd(nc, in_maps, core_ids=core_ids)
    else:
        res = run_bass_kernel_spmd(nc, in_maps, core_ids=core_ids)
    LAST_EXEC_NS = res.exec_time_ns
    return assemble(res.results, prep)


# revision 35
# speedup vs baseline: 1.2148x; 1.2148x over previous
"""Trainium2 Bass kernel for nn_AgentRNN: dual GRU scan + subtask/Q heads.

Sharding (8 cores, data-parallel, no collectives):
  - GRU2/Q path ("branch 2"): core c owns agent n=c -> rows [128c, 128(c+1))
    of the NB=1024 flat batch (nb-major).
  - GRU1/logits path ("branch 1"): core c owns batch columns b in
    [16c, 16(c+1)) across all 8 agents, with rows ordered b-major
    (local row j <-> (b = 16c + j//8, n = j%8)).
    With this choice, prob_flat row i == q row i live on the same core
    (prob_flat maps i -> (b=i//8, n=i%8), i.e. j == i - 128c), so the
    q_vals one-hot gather needs no cross-core traffic.

Precision:
  - Branch 1 feeds an argmax over logits whose top-2 gaps go down to ~3e-9,
    so its whole chain (embed, gi, gh) runs as 3-term bf16 split matmuls
    (x ~ xh+xl, W ~ Wh+Wl; terms xh@Wh + xl@Wh + xh@Wl), giving ~2^-17
    effective input precision at 3 bf16 matmul passes (vs 4 for true fp32).
    The logits head itself (y1 @ W_le) runs in true fp32.
  - Branch 2 only feeds Q *values* (no selection), so single-term bf16.

Device layout per core (R=128 rows per branch):
  - batch layout: partitions = local rows (128), free = H
  - T layout:     tiles (128, 512) where column block j holds
                  h[:, 128j:128(j+1)].T  (matmul lhsT operands)
"""
import sys
import os

sys.path.insert(0, "/opt/trn_rl_repo")

import numpy as np
import ml_dtypes

BF = ml_dtypes.bfloat16

T, N, B, D = 64, 8, 128, 128
H, S, K, A = 512, 64, 16, 16
NB = N * B
R = 128          # rows per core per branch
NCORES = 8
HC = H // 128    # 4 h-chunks

_CACHE = {}


def _build_program(cfg):
    """cfg: (bi1_nz, bhn1_nz, bi2_nz, bhn2_nz, ble_nz, bemb1_nz, t_steps)"""
    import concourse.bacc as bacc
    import concourse.tile as tile
    from concourse import mybir

    bi_nz = {1: cfg[0], 2: cfg[2]}
    bhn_nz = {1: cfg[1], 2: cfg[3]}
    ble_nz = cfg[4]
    bemb1_nz = cfg[5]
    h0z = cfg[6]
    TS = cfg[7]

    f32 = mybir.dt.float32
    bf16 = mybir.dt.bfloat16
    AF = mybir.ActivationFunctionType
    ALU = mybir.AluOpType
    AX = mybir.AxisListType

    nc = bacc.Bacc("TRN2", num_devices=NCORES)

    din = {}

    def inp(name, shape, dt):
        din[name] = nc.dram_tensor(name, list(shape), dt, kind="ExternalInput")
        return din[name]

    # branch 1 (split operands), branch 2 (single bf16)
    inp("obsT1h", (TS, D, R), bf16)
    inp("obsT1l", (TS, D, R), bf16)
    inp("obsT2", (TS, D, R), bf16)
    for sfx in ("h", "l"):
        inp(f"wi1{sfx}", (128, HC * 3 * H), bf16)
        inp(f"wh1{sfx}", (128, HC * 3 * H), bf16)
        inp(f"wembed{sfx}", (128, H), bf16)
    inp("wi2", (128, HC * 3 * H), bf16)
    inp("wh2", (128, HC * 3 * H), bf16)
    inp("wpol", (128, H), bf16)
    for br in (1, 2):
        inp(f"ndt{br}", (128, TS), f32)
        inp(f"ndb{br}", (TS, R), bf16)
        inp(f"h{br}b0", (R, H), f32)
    inp("h1t0h", (128, H), bf16)
    inp("h1t0l", (128, H), bf16)
    inp("h2t0", (128, H), bf16)
    inp("bemb", (128, HC), f32)
    inp("bpol", (128, HC), f32)
    inp("w1f", (128, HC * K * A), bf16)
    inp("wle", (128, HC * K), f32)
    inp("b1f", (1, K * A), bf16)
    inp("onesb", (1, R), bf16)
    inp("onesf", (1, R), f32)
    inp("ident", (128, 128), f32)
    if bi_nz[1]:
        inp("bi1h", (1, 3 * H), bf16)
        inp("bi1l", (1, 3 * H), bf16)
    if bhn_nz[1]:
        inp("bhn1h", (1, H), bf16)
        inp("bhn1l", (1, H), bf16)
    if bi_nz[2]:
        inp("bi2", (1, 3 * H), bf16)
    if bhn_nz[2]:
        inp("bhn2", (1, H), bf16)
    if ble_nz:
        inp("ble", (1, K), f32)

    out_h1 = nc.dram_tensor("h1f", [R, H], f32, kind="ExternalOutput")
    out_h2 = nc.dram_tensor("h2f", [R, H], f32, kind="ExternalOutput")
    out_lg = nc.dram_tensor("lg1", [R, TS * K], f32, kind="ExternalOutput")
    out_qv = nc.dram_tensor("qv2", [R, TS * A], f32, kind="ExternalOutput")

    with tile.TileContext(nc) as tc:
        _emit(nc, tc, din, out_h1, out_h2, out_lg, out_qv,
              bi_nz, bhn_nz, ble_nz, bemb1_nz, h0z, TS, f32, bf16, AF, ALU, AX)
    nc.compile()
    return nc


def _dedup_ldweights(nc):
    """Remove InstLdweights that reload the exact weights already resident:
    consecutive-on-the-PE-stream LDWs with an identical stationary AP (only
    non-transpose matmuls in between) are redundant -- the PE array still
    holds the data.  Waits from a removed LDW migrate to the next matmul."""
    from concourse import mybir

    removed = kept = 0
    for fn in nc.m.functions:
        for blk in fn.blocks:
            new_insts = []
            last_sig = None
            pending_waits = []
            for inst in blk.instructions:
                tn = type(inst).__name__
                if tn == "InstLdweights":
                    ap = inst.ins[-1]
                    sig = (ap.memref, ap.offset,
                           tuple(tuple(p) for p in ap.ap), str(ap.dtype),
                           inst.perf_mode, inst.is_transpose,
                           inst.tile_position)
                    if sig == last_sig:
                        removed += 1
                        si = inst.sync_info
                        if si is not None:
                            pending_waits.extend(si.on_wait)
                            assert not si.on_update, \
                                "removed LDW carries sem updates"
                        continue
                    last_sig = sig
                    kept += 1
                elif tn == "InstMatmult":
                    if pending_waits:
                        si = inst.sync_info
                        if si is None:
                            inst.sync_info = mybir.SyncInfo(
                                on_wait=list(pending_waits), on_update=[])
                        else:
                            have = {(w.id, w.wait_value) for w in si.on_wait}
                            for w in pending_waits:
                                if (w.id, w.wait_value) not in have:
                                    si.on_wait.append(w)
                        pending_waits = []
                new_insts.append(inst)
            assert not pending_waits, "dangling waits from removed LDW"
            blk.instructions[:] = new_insts
    print(f"[kernel] ldweights dedup: removed {removed}, kept {kept}")
    return nc


def _emit(nc, tc, din, out_h1, out_h2, out_lg, out_qv,
          bi_nz, bhn_nz, ble_nz, bemb1_nz, h0z, TS, f32, bf16, AF, ALU, AX):
    from contextlib import ExitStack

    ctx = ExitStack()
    with ctx:
        const = ctx.enter_context(tc.tile_pool(name="const", bufs=1))
        obs_p = ctx.enter_context(tc.tile_pool(name="obs", bufs=3))
        ae_p = ctx.enter_context(tc.tile_pool(name="aeT", bufs=4))
        st_p = ctx.enter_context(tc.tile_pool(name="state", bufs=2))
        tmp_p = ctx.enter_context(tc.tile_pool(name="tmp", bufs=8))
        outp = ctx.enter_context(tc.tile_pool(name="outs", bufs=1))
        ps_g = ctx.enter_context(tc.tile_pool(name="psg", bufs=4, space="PSUM"))
        ps_tr = ctx.enter_context(tc.tile_pool(name="pstr", bufs=2, space="PSUM"))
        ps_m = ctx.enter_context(tc.tile_pool(name="psm", bufs=2, space="PSUM"))

        def ctile(name, dt):
            t_ = const.tile(list(din[name].shape), dt, tag=name, name=name)
            nc.sync.dma_start(t_[:], din[name].ap())
            return t_

        # small embed-critical tiles first so the first obs/embed DMAs are
        # not queued behind ~10MB of GRU weights (PE can start ~40us earlier);
        # the big wi/wh DMAs are issued after the first embed groups.
        wi = {}
        wh = {}
        wemb = {1: {"h": ctile("wembedh", bf16), "l": ctile("wembedl", bf16)},
                2: {"h": ctile("wpol", bf16)}}
        # (data_sfx, weight_sfx) term lists
        terms = {1: (("h", "h"), ("l", "h"), ("h", "l")), 2: (("h", "h"),)}

        bemb = {1: ctile("bemb", f32), 2: ctile("bpol", f32)}
        ndt = {1: ctile("ndt1", f32), 2: ctile("ndt2", f32)}
        ndb_t = {}  # (gidx, br) -> (1, ng*R) sbuf tile, loaded with the group
        w1f = ctile("w1f", bf16)
        wle = ctile("wle", f32)
        b1f = ctile("b1f", bf16)
        onesb = ctile("onesb", bf16)
        onesf = ctile("onesf", f32)
        ident = ctile("ident", f32)
        bi = {}
        bhn = {}
        if bi_nz[1]:
            bi[1] = {"h": ctile("bi1h", bf16), "l": ctile("bi1l", bf16)}
        if bhn_nz[1]:
            bhn[1] = {"h": ctile("bhn1h", bf16), "l": ctile("bhn1l", bf16)}
        if bi_nz[2]:
            bi[2] = {"h": ctile("bi2", bf16)}
        if bhn_nz[2]:
            bhn[2] = {"h": ctile("bhn2", bf16)}
        ble = ctile("ble", f32) if ble_nz else None

        hB = {br: ctile(f"h{br}b0", f32) for br in (1, 2)}
        hT = {1: {"h": ctile("h1t0h", bf16), "l": ctile("h1t0l", bf16)},
              2: {"h": ctile("h2t0", bf16)}}

        # b1f broadcast across partitions, built once (rank-1 ones matmul)
        b1p = ps_m.tile([128, K * A], f32, tag="m", name="b1p")
        nc.tensor.matmul(b1p[:], onesb[:, :], b1f[:], start=True, stop=True)
        b1bc = outp.tile([128, K * A], f32, tag="b1bc")
        nc.vector.tensor_copy(b1bc[:], b1p[:])

        oh_sb = outp.tile([128, TS * K], f32, tag="oh")
        lg_sb = outp.tile([128, TS * K], f32, tag="lg")
        qv_sb = outp.tile([128, TS * A], f32, tag="qv")

        aeT = {1: {}, 2: {}}
        grp_w = {}

        aeT_groups = {}

        def emit_embed(gidx0, br):
            t0 = 4 * gidx0
            if t0 >= TS:
                return
            ng_t = min(4, TS - t0)  # timesteps in this obs group
            obs_d = {}
            for dsfx in ("h", "l") if br == 1 else ("h",):
                nm = f"obsT{br}{dsfx}" if br == 1 else "obsT2"
                ob = obs_p.tile([128, ng_t * R], bf16,
                                tag=f"ob{br}{dsfx}", name=f"ob{br}{dsfx}")
                nc.sync.dma_start(
                    ob[:].rearrange("d (t b) -> d t b", t=ng_t),
                    din[nm].ap()[t0:t0 + ng_t].rearrange("t d b -> d t b"))
                obs_d[dsfx] = ob
            nb = obs_p.tile([1, ng_t * R], bf16, tag=f"nb{br}", name=f"nb{br}",
                            bufs=3)
            nc.sync.dma_start(
                nb[:],
                din[f"ndb{br}"].ap().rearrange("(o t) b -> o (t b)", o=1)[
                    0:1, t0 * R:(t0 + ng_t) * R])
            ndBp4 = ps_tr.tile([128, ng_t * R], f32, tag="tr", name="ndBp4")
            nc.tensor.matmul(ndBp4[:], onesb[:, :], nb[:], start=True, stop=True)
            nd4 = st_p.tile([128, ng_t * R], f32, tag=f"nd4{br}", name=f"nd4{br}",
                            bufs=3)
            nc.vector.tensor_copy(nd4[:], ndBp4[:])
            ndb_t[(gidx0, br)] = nd4
            emb_terms = {1: (("h", "h"), ("l", "h"), ("h", "l")),
                         2: (("h", "h"),)}[br]
            if br == 1:
                ae1h = ae_p.tile([128, HC * ng_t * R], bf16, tag="ae1h",
                                 name="ae1h", bufs=3)
                ae1l = ae_p.tile([128, HC * ng_t * R], bf16, tag="ae1l",
                                 name="ae1l", bufs=3)
            else:
                ae2 = ae_p.tile([128, HC * ng_t * R], bf16, tag="ae2",
                                name="ae2", bufs=3)
            for hc in range(HC):
                pa = ps_m.tile([128, ng_t * R], f32, tag="m", name="pa")
                for i, (ds, ws) in enumerate(emb_terms):
                    nc.tensor.matmul(
                        pa[:], wemb[br][ws][:, hc * 128:(hc + 1) * 128],
                        obs_d[ds][:], start=(i == 0),
                        stop=(i == len(emb_terms) - 1), skip_group_check=True)
                sl = slice(hc * ng_t * R, (hc + 1) * ng_t * R)
                if br == 2:
                    nc.scalar.activation(ae2[:, sl], pa[:], AF.Relu,
                                         bias=bemb[2][:, hc:hc + 1])
                elif bemb1_nz:
                    aef = tmp_p.tile([128, ng_t * R], f32, tag="aef",
                                     bufs=1, name="aef")
                    nc.scalar.activation(aef[:], pa[:], AF.Relu,
                                         bias=bemb[1][:, hc:hc + 1])
                    nc.vector.tensor_copy(ae1h[:, sl], aef[:])
                    nc.vector.tensor_sub(ae1l[:, sl], aef[:], ae1h[:, sl])
                else:
                    nc.scalar.activation(ae1h[:, sl], pa[:], AF.Relu)
                    # ael = relu(ps) - aeh   (one fused DVE op)
                    nc.vector.scalar_tensor_tensor(
                        ae1l[:, sl], pa[:], 0.0, ae1h[:, sl],
                        ALU.max, ALU.subtract)
            if br == 1:
                aeT_groups[(gidx0, 1)] = ({"h": ae1h, "l": ae1l}, ng_t * R)
            else:
                aeT_groups[(gidx0, 2)] = ({"h": ae2}, ng_t * R)

        for brr in (1, 2):
            emit_embed(0, brr)
        wi[1] = {"h": ctile("wi1h", bf16), "l": ctile("wi1l", bf16)}
        wh[1] = {"h": ctile("wh1h", bf16), "l": ctile("wh1l", bf16)}
        wi[2] = {"h": ctile("wi2", bf16)}
        wh[2] = {"h": ctile("wh2", bf16)}
        for brr in (1, 2):
            emit_embed(1, brr)
        for t in range(TS):
            if t % 4 == 0:
                for brr in (1, 2):
                    aeT[brr], grp_w[brr] = aeT_groups.pop((t // 4, brr))
            tl = t % 4

            gidx = {"r": 0, "z": 1, "inn": 2, "hn": 2}

            def mm_phase(br):
                # gi + gh accumulated per gate bank; phases ordered so r/z
                # close early; kc-major, lhsT-grouped for ldweights dedup.
                g = {gate: ps_g.tile([128, 512], f32, tag="g", name=f"g{gate}")
                     for gate in ("r", "z", "hn", "inn")}
                dws = {}
                for ds, ws in terms[br]:
                    dws.setdefault(ds, []).append(ws)

                def gi_lhs(kc, ds):
                    return aeT[br][ds][:, kc * grp_w[br] + tl * R:
                                       kc * grp_w[br] + tl * R + R]

                def gh_lhs(kc, ds):
                    return hT[br][ds][:, kc * 128:(kc + 1) * 128]

                def wslice(w, ws, kc, gate):
                    return w[ws][:, kc * 1536 + gidx[gate] * 512:
                                 kc * 1536 + gidx[gate] * 512 + 512]

                skip_gh = h0z and t == 0
                sched = []  # (bank, lhsT_ap, rhs_ap)
                for kc in range(HC):
                    for ds, wss in dws.items():
                        for ws in wss:
                            for gate in ("r", "z"):
                                sched.append((gate, gi_lhs(kc, ds),
                                              wslice(wi[br], ws, kc, gate)))
                if bi_nz[br]:
                    for ws in bi[br]:
                        for gate in ("r", "z"):
                            sched.append((gate, onesb[:, :],
                                          bi[br][ws][:, gidx[gate] * 512:
                                                     gidx[gate] * 512 + 512]))
                if not skip_gh:
                    for kc in range(HC):
                        for ds, wss in dws.items():
                            for ws in wss:
                                for gate in ("r", "z"):
                                    sched.append((gate, gh_lhs(kc, ds),
                                                  wslice(wh[br], ws, kc, gate)))
                    for kc in range(HC):
                        for ds, wss in dws.items():
                            for ws in wss:
                                sched.append(("hn", gh_lhs(kc, ds),
                                              wslice(wh[br], ws, kc, "hn")))
                if bhn_nz[br]:
                    for ws in bhn[br]:
                        sched.append(("hn", onesb[:, :], bhn[br][ws][:, :]))
                for kc in range(HC):
                    for ds, wss in dws.items():
                        for ws in wss:
                            sched.append(("inn", gi_lhs(kc, ds),
                                          wslice(wi[br], ws, kc, "inn")))
                if bi_nz[br]:
                    for ws in bi[br]:
                        sched.append(("inn", onesb[:, :],
                                      bi[br][ws][:, gidx["inn"] * 512:
                                                 gidx["inn"] * 512 + 512]))
                total = {}
                for gate, _, _ in sched:
                    total[gate] = total.get(gate, 0) + 1
                seen = {gate: 0 for gate in total}
                for gate, lhs, rhs in sched:
                    seen[gate] += 1
                    nc.tensor.matmul(g[gate][:], lhs, rhs,
                                     start=(seen[gate] == 1),
                                     stop=(seen[gate] == total[gate]),
                                     skip_group_check=True)
                return g

            def gates_tail(br, g):
                skip_gh = h0z and t == 0
                zg = tmp_p.tile([128, 512], f32, tag="tmp", bufs=7, name="zg")
                nc.scalar.activation(zg[:], g["z"][:], AF.Sigmoid)
                ngt = tmp_p.tile([128, 512], f32, tag="tmp", bufs=7, name="ngt")
                if skip_gh:
                    nc.scalar.activation(ngt[:], g["inn"][:], AF.Tanh)
                else:
                    rg = tmp_p.tile([128, 512], f32, tag="tmp", bufs=7, name="rg")
                    nc.scalar.activation(rg[:], g["r"][:], AF.Sigmoid)
                    m = tmp_p.tile([128, 512], f32, tag="tmp", bufs=7, name="m")
                    nc.vector.tensor_tensor(m[:], rg[:], g["hn"][:], ALU.mult)
                    nin = tmp_p.tile([128, 512], f32, tag="tmp", bufs=7, name="nin")
                    nc.vector.tensor_tensor(nin[:], m[:], g["inn"][:], ALU.add)
                    nc.scalar.activation(ngt[:], nin[:], AF.Tanh)
                d = tmp_p.tile([128, 512], f32, tag="tmp", bufs=7, name="d")
                nc.vector.tensor_sub(d[:], hB[br][:], ngt[:])
                p_ = tmp_p.tile([128, 512], f32, tag="tmp", bufs=7, name="p_")
                nc.vector.tensor_mul(p_[:], zg[:], d[:])
                hnew = tmp_p.tile([128, 512], f32, tag="hnew", bufs=3, name="hnew")
                nc.vector.tensor_add(hnew[:], ngt[:], p_[:])
                if t == TS - 1:
                    nc.sync.dma_start((out_h1 if br == 1 else out_h2).ap(),
                                      hnew[:])
                # masked next-state (batch layout)
                hBn = st_p.tile([128, 512], f32, tag=f"hB{br}", name=f"hB{br}")
                nc.vector.tensor_scalar_mul(hBn[:], hnew[:], ndt[br][:, t:t + 1])
                hB[br] = hBn
                # single transpose set (unmasked); T-layout mask comes from a
                # rank-1 ones x nd-row matmul broadcast across partitions
                tru = ps_tr.tile([128, 512], f32, tag="tr", name="tru")
                for j in range(4):
                    nc.tensor.transpose(tru[:, j * 128:(j + 1) * 128],
                                        hnew[:, j * 128:(j + 1) * 128], ident[:])
                ndB3 = ndb_t[(t // 4, br)][:, tl * R:(tl + 1) * R].rearrange(
                    "p (o b) -> p o b", o=1).to_broadcast((128, 4, 128))
                tru3 = tru[:].rearrange("p (j b) -> p j b", j=4)
                if br == 1:
                    t1 = tmp_p.tile([128, 512], f32, tag="tmp", bufs=7, name="t1")
                    nc.vector.tensor_tensor(
                        t1[:].rearrange("p (j b) -> p j b", j=4), tru3, ndB3,
                        ALU.mult)
                    hTh = st_p.tile([128, 512], bf16, tag="hT1h", name="hT1h")
                    nc.scalar.activation(hTh[:], t1[:], AF.Copy)
                    hTl = st_p.tile([128, 512], bf16, tag="hT1l", name="hT1l")
                    nc.vector.tensor_sub(hTl[:], t1[:], hTh[:])
                    hT[1] = {"h": hTh, "l": hTl}
                else:
                    hT2n = st_p.tile([128, 512], bf16, tag="hT2", name="hT2")
                    nc.vector.tensor_tensor(
                        hT2n[:].rearrange("p (j b) -> p j b", j=4), tru3, ndB3,
                        ALU.mult)
                    hT[2] = {"h": hT2n}

                if br == 1:
                    # logits in true fp32 + one-hot
                    yT = st_p.tile([128, 512], f32, tag="yT1", name="yT1")
                    nc.scalar.activation(yT[:], tru[:], AF.Copy)
                    lgp = ps_m.tile([128, 512], f32, tag="m", name="lgp")
                    lmms = [(yT[:, kc * 128:(kc + 1) * 128],
                             wle[:, kc * K:(kc + 1) * K]) for kc in range(HC)]
                    if ble_nz:
                        lmms.append((onesf[:, :], ble[:, :]))
                    for i, (lhs, rhs) in enumerate(lmms):
                        nc.tensor.matmul(lgp[:, 0:K], lhs, rhs, start=(i == 0),
                                         stop=(i == len(lmms) - 1))
                    nc.vector.tensor_copy(lg_sb[:, t * K:(t + 1) * K], lgp[:, 0:K])
                    mx = tmp_p.tile([128, 1], f32, tag="mx", bufs=2, name="mx")
                    nc.vector.tensor_reduce(mx[:], lgp[:, 0:K], AX.X, ALU.max)
                    nc.vector.tensor_tensor(oh_sb[:, t * K:(t + 1) * K], lgp[:, 0:K],
                                            mx[:, 0:1].to_broadcast((128, K)),
                                            ALU.is_ge)
                else:
                    # q = y2 @ w1f + b1f ; q_vals = sum_k OH * q
                    yT = st_p.tile([128, 512], bf16, tag="yT2", name="yT2")
                    nc.scalar.activation(yT[:], tru[:], AF.Copy)
                    qp = ps_m.tile([128, 512], f32, tag="m", name="qp")
                    qmms = [(yT[:, kc * 128:(kc + 1) * 128],
                             w1f[:, kc * K * A:(kc + 1) * K * A]) for kc in range(HC)]
                    for i, (lhs, rhs) in enumerate(qmms):
                        nc.tensor.matmul(qp[:, 0:K * A], lhs, rhs, start=(i == 0),
                                         stop=(i == len(qmms) - 1))
                    qb = tmp_p.tile([128, K * A], f32, tag="qm", bufs=4, name="qb")
                    nc.vector.tensor_tensor(qb[:], qp[:, 0:K * A], b1bc[:], ALU.add)
                    qm = tmp_p.tile([128, K * A], f32, tag="qm", bufs=4, name="qm")
                    nc.vector.tensor_tensor(
                        qm[:].rearrange("p (a k) -> p a k", k=K),
                        qb[:].rearrange("p (a k) -> p a k", k=K),
                        oh_sb[:, t * K:(t + 1) * K].rearrange(
                            "p (o k) -> p o k", o=1).to_broadcast((128, A, K)),
                        ALU.mult)
                    nc.vector.tensor_reduce(qv_sb[:, t * A:(t + 1) * A],
                                            qm[:].rearrange("p (a k) -> p a k", k=K),
                                            AX.X, ALU.add)

            g1 = mm_phase(1)
            g2 = mm_phase(2)
            gates_tail(1, g1)
            if t % 4 == 0:
                emit_embed(t // 4 + 2, 1)
            elif t % 4 == 2:
                emit_embed(t // 4 + 2, 2)
            gates_tail(2, g2)

        nc.sync.dma_start(out_lg.ap(), lg_sb[:])
        nc.sync.dma_start(out_qv.ap(), qv_sb[:])


def _bf16_split(x):
    xh = x.astype(BF)
    xl = (x - xh.astype(np.float32)).astype(BF)
    return xh, xl


def _host_prep(inputs):
    f = lambda x: np.ascontiguousarray(np.asarray(x), dtype=np.float32)
    W_embed, b_embed = f(inputs["W_embed"]), f(inputs["b_embed"])
    Wi1, bi1 = f(inputs["Wi1"]), f(inputs["bi1"])
    Wh1, bhn1 = f(inputs["Wh1"]), f(inputs["bhn1"])
    W_sub, b_sub = f(inputs["W_sub"]), f(inputs["b_sub"])
    W_e1, b_e1 = f(inputs["W_e1"]), f(inputs["b_e1"])
    W_e2, b_e2 = f(inputs["W_e2"]), f(inputs["b_e2"])
    W_pol, b_pol = f(inputs["W_pol"]), f(inputs["b_pol"])
    Wi2, bi2 = f(inputs["Wi2"]), f(inputs["bi2"])
    Wh2, bhn2 = f(inputs["Wh2"]), f(inputs["bhn2"])
    W_w1, b_w1 = f(inputs["W_w1"]), f(inputs["b_w1"])
    W_b1, b_b1 = f(inputs["W_b1"]), f(inputs["b_b1"])

    e = np.tanh(np.maximum(W_e1 + b_e1, 0.0) @ W_e2 + b_e2)      # (K, S)
    W_le = W_sub @ e.T                                           # (H, K)
    b_le = b_sub @ e.T                                           # (K,)
    w1 = (e @ W_w1 + b_w1).reshape(K, H, A)                      # (K, H, A)
    b1 = e @ W_b1 + b_b1                                         # (K, A)
    # q matmul columns ordered (a, k): col a*K + k = w1[k, :, a]
    W1f = np.ascontiguousarray(w1.transpose(2, 0, 1).reshape(K * A, H).T)
    b1f = np.ascontiguousarray(b1.T.reshape(1, K * A))

    blk = lambda M: np.ascontiguousarray(
        np.concatenate([M[kc * 128:(kc + 1) * 128, :]
                        for kc in range(M.shape[0] // 128)], axis=1))
    wi1h, wi1l = _bf16_split(blk(Wi1))
    wh1h, wh1l = _bf16_split(blk(Wh1))
    wembedh, wembedl = _bf16_split(W_embed)
    prep = dict(
        wi1h=wi1h, wi1l=wi1l, wh1h=wh1h, wh1l=wh1l,
        wembedh=wembedh, wembedl=wembedl,
        wi2=blk(Wi2).astype(BF), wh2=blk(Wh2).astype(BF),
        wpol=W_pol.astype(BF),
        bemb=np.ascontiguousarray(b_embed.reshape(HC, 128).T),
        bpol=np.ascontiguousarray(b_pol.reshape(HC, 128).T),
        w1f=blk(W1f).astype(BF), wle=blk(W_le),
        b1f=b1f.astype(BF),
        onesb=np.ones((1, R), BF), onesf=np.ones((1, R), np.float32),
        ident=np.eye(128, dtype=np.float32),
        e=e,
    )
    flags = dict(
        bi1_nz=bool(np.any(bi1)), bhn1_nz=bool(np.any(bhn1)),
        bi2_nz=bool(np.any(bi2)), bhn2_nz=bool(np.any(bhn2)),
        ble_nz=bool(np.any(b_le)), bemb1_nz=bool(np.any(b_embed)),
        h0z=not (np.any(np.asarray(inputs["h1"])) or
                 np.any(np.asarray(inputs["h2"]))),
    )
    if flags["bi1_nz"]:
        prep["bi1h"], prep["bi1l"] = _bf16_split(bi1.reshape(1, 3 * H))
    if flags["bhn1_nz"]:
        prep["bhn1h"], prep["bhn1l"] = _bf16_split(bhn1.reshape(1, H))
    if flags["bi2_nz"]:
        prep["bi2"] = bi2.reshape(1, 3 * H).astype(BF)
    if flags["bhn2_nz"]:
        prep["bhn2"] = bhn2.reshape(1, H).astype(BF)
    if flags["ble_nz"]:
        prep["ble"] = b_le.reshape(1, K)
    return prep, flags


def _tlayout(h):
    """(R, H) batch-layout -> (128, H) T-layout (column block j = chunk j^T)."""
    return np.ascontiguousarray(
        h.T.reshape(HC, 128, R).transpose(1, 0, 2).reshape(128, HC * R))


def _core_inputs(inputs, prep, flags, c, t_steps):
    obs = np.asarray(inputs["obs"], dtype=np.float32)
    done = np.asarray(inputs["done"])
    nd = 1.0 - done.astype(np.float32)          # (T, NB)
    h1 = np.asarray(inputs["h1"], dtype=np.float32)
    h2 = np.asarray(inputs["h2"], dtype=np.float32)

    m = {}
    # branch 2: agent shard, rows nb-major
    rows2 = slice(c * R, (c + 1) * R)
    obs2 = np.ascontiguousarray(obs[:, c].transpose(0, 2, 1))[:t_steps]
    m["obsT2"] = obs2.astype(BF)
    nd2 = nd[:, rows2]                          # (T, R)
    ndn2 = np.vstack([nd2[1:], np.ones((1, R), np.float32)])
    m["ndt2"] = np.ascontiguousarray(ndn2.T)[:, :t_steps]
    m["ndb2"] = ndn2[:t_steps].astype(BF)
    h2m = h2[rows2] * nd2[0][:, None]
    m["h2b0"] = np.ascontiguousarray(h2m)
    m["h2t0"] = _tlayout(h2m).astype(BF)

    # branch 1: b-slice shard, rows b-major: local j = r_*8 + n, b = 16c + r_
    bsl = slice(16 * c, 16 * (c + 1))
    Tn = obs.shape[0]
    obs1 = obs[:, :, bsl, :].transpose(0, 2, 1, 3).reshape(Tn, R, D)
    obs1T = np.ascontiguousarray(obs1.transpose(0, 2, 1))[:t_steps]
    m["obsT1h"], m["obsT1l"] = _bf16_split(obs1T)
    done_v = done.reshape(Tn, N, B)
    nd1 = (1.0 - done_v[:, :, bsl].astype(np.float32)).transpose(0, 2, 1).reshape(Tn, R)
    ndn1 = np.vstack([nd1[1:], np.ones((1, R), np.float32)])
    m["ndt1"] = np.ascontiguousarray(ndn1.T)[:, :t_steps]
    m["ndb1"] = ndn1[:t_steps].astype(BF)
    h1_v = h1.reshape(N, B, H)[:, bsl].transpose(1, 0, 2).reshape(R, H)
    h1m = h1_v * nd1[0][:, None]
    m["h1b0"] = np.ascontiguousarray(h1m)
    m["h1t0h"], m["h1t0l"] = _bf16_split(_tlayout(h1m))

    for k in ("wi1h", "wi1l", "wh1h", "wh1l", "wembedh", "wembedl",
              "wi2", "wh2", "wpol", "bemb", "bpol", "w1f", "wle", "b1f",
              "onesb", "onesf", "ident"):
        m[k] = prep[k]
    for k, fl in (("bi1h", "bi1_nz"), ("bi1l", "bi1_nz"), ("bi2", "bi2_nz"),
                  ("bhn1h", "bhn1_nz"), ("bhn1l", "bhn1_nz"),
                  ("bhn2", "bhn2_nz"), ("ble", "ble_nz")):
        if flags[fl]:
            m[k] = prep[k]
    return m


def get_program(flags, t_steps=T):
    key = (flags["bi1_nz"], flags["bhn1_nz"], flags["bi2_nz"], flags["bhn2_nz"],
           flags["ble_nz"], flags["bemb1_nz"], flags["h0z"], t_steps)
    if key not in _CACHE:
        _CACHE[key] = _build_program(key)
    return _CACHE[key]


def assemble(results, prep, t_steps=T):
    """Gather per-core results into the full output pytree."""
    h1f = np.empty((NB, H), np.float32)
    h2f = np.empty((NB, H), np.float32)
    qv = np.empty((t_steps, NB, A), np.float32)
    lg = np.empty((t_steps, B, N, K), np.float32)
    for c in range(NCORES):
        r = results[c]
        h2f[c * R:(c + 1) * R] = r["h2f"]
        qv[:, c * R:(c + 1) * R] = r["qv2"].reshape(R, t_steps, A).transpose(1, 0, 2)
        # branch1 rows: j = r_*8 + n  -> (b = 16c + r_, n)
        l_ = r["lg1"].reshape(16, 8, t_steps, K)      # (r_, n, t, k)
        lg[:, 16 * c:16 * (c + 1)] = l_.transpose(2, 0, 1, 3)
        h1_ = r["h1f"].reshape(16, 8, H)              # (r_, n, H)
        h1f.reshape(N, B, H)[:, 16 * c:16 * (c + 1)] = h1_.transpose(1, 0, 2)
    se = np.broadcast_to(prep["e"], (t_steps, B, K, S)).copy()
    return ((h1f, h2f), qv, lg, se)


LAST_EXEC_NS = None


def kernel(_trace=False, **inputs):
    global LAST_EXEC_NS
    from concourse.bass_utils import run_bass_kernel_spmd

    prep, flags = _host_prep(inputs)
    nc = get_program(flags)
    in_maps = [_core_inputs(inputs, prep, flags, c, T) for c in range(NCORES)]
    core_ids = list(range(NCORES))
    if _trace:
        try:
            res = run_bass_kernel_spmd(nc, in_maps, core_ids=core_ids, trace=True)
        except Exception as e:  # trace infra (NTFF hook / upload) can fail
            print(f"[kernel] trace run failed ({e!r}); rerunning untraced")
            res = run_bass_kernel_spmd(nc, in_maps, core_ids=core_ids)
    else:
        res = run_bass_kernel_spmd(nc, in_maps, core_ids=core_ids)
    LAST_EXEC_NS = res.exec_time_ns
    return assemble(res.results, prep)


# revision 36
# speedup vs baseline: 1.2299x; 1.0124x over previous
"""Trainium2 Bass kernel for nn_AgentRNN: dual GRU scan + subtask/Q heads.

Sharding (8 cores, data-parallel, no collectives):
  - GRU2/Q path ("branch 2"): core c owns agent n=c -> rows [128c, 128(c+1))
    of the NB=1024 flat batch (nb-major).
  - GRU1/logits path ("branch 1"): core c owns batch columns b in
    [16c, 16(c+1)) across all 8 agents, with rows ordered b-major
    (local row j <-> (b = 16c + j//8, n = j%8)).
    With this choice, prob_flat row i == q row i live on the same core
    (prob_flat maps i -> (b=i//8, n=i%8), i.e. j == i - 128c), so the
    q_vals one-hot gather needs no cross-core traffic.

Precision:
  - Branch 1 feeds an argmax over logits whose top-2 gaps go down to ~3e-9,
    so its whole chain (embed, gi, gh) runs as 3-term bf16 split matmuls
    (x ~ xh+xl, W ~ Wh+Wl; terms xh@Wh + xl@Wh + xh@Wl), giving ~2^-17
    effective input precision at 3 bf16 matmul passes (vs 4 for true fp32).
    The logits head itself (y1 @ W_le) runs in true fp32.
  - Branch 2 only feeds Q *values* (no selection), so single-term bf16.

Device layout per core (R=128 rows per branch):
  - batch layout: partitions = local rows (128), free = H
  - T layout:     tiles (128, 512) where column block j holds
                  h[:, 128j:128(j+1)].T  (matmul lhsT operands)
"""
import sys
import os

sys.path.insert(0, "/opt/trn_rl_repo")

import numpy as np
import ml_dtypes

BF = ml_dtypes.bfloat16

T, N, B, D = 64, 8, 128, 128
H, S, K, A = 512, 64, 16, 16
NB = N * B
R = 128          # rows per core per branch
NCORES = 8
HC = H // 128    # 4 h-chunks

_CACHE = {}


def _build_program(cfg):
    """cfg: (bi1_nz, bhn1_nz, bi2_nz, bhn2_nz, ble_nz, bemb1_nz, t_steps)"""
    import concourse.bacc as bacc
    import concourse.tile as tile
    from concourse import mybir

    bi_nz = {1: cfg[0], 2: cfg[2]}
    bhn_nz = {1: cfg[1], 2: cfg[3]}
    ble_nz = cfg[4]
    bemb1_nz = cfg[5]
    h0z = cfg[6]
    TS = cfg[7]

    f32 = mybir.dt.float32
    bf16 = mybir.dt.bfloat16
    AF = mybir.ActivationFunctionType
    ALU = mybir.AluOpType
    AX = mybir.AxisListType

    nc = bacc.Bacc("TRN2", num_devices=NCORES)

    din = {}

    def inp(name, shape, dt):
        din[name] = nc.dram_tensor(name, list(shape), dt, kind="ExternalInput")
        return din[name]

    # branch 1 (split operands), branch 2 (single bf16)
    inp("obsT1h", (TS, D, R), bf16)
    inp("obsT1l", (TS, D, R), bf16)
    inp("obsT2", (TS, D, R), bf16)
    for sfx in ("h", "l"):
        inp(f"wi1{sfx}", (128, HC * 3 * H), bf16)
        inp(f"wh1{sfx}", (128, HC * 3 * H), bf16)
        inp(f"wembed{sfx}", (128, H), bf16)
    inp("wi2", (128, HC * 3 * H), bf16)
    inp("wh2", (128, HC * 3 * H), bf16)
    inp("wpol", (128, H), bf16)
    for br in (1, 2):
        inp(f"ndt{br}", (128, TS), f32)
        inp(f"ndb{br}", (TS, R), bf16)
        inp(f"h{br}b0", (R, H), f32)
    inp("h1t0h", (128, H), bf16)
    inp("h1t0l", (128, H), bf16)
    inp("h2t0", (128, H), bf16)
    inp("bemb", (128, HC), f32)
    inp("bpol", (128, HC), f32)
    inp("w1f", (128, HC * K * A), bf16)
    inp("wle", (128, HC * K), f32)
    inp("b1f", (1, K * A), bf16)
    inp("onesb", (1, R), bf16)
    inp("onesf", (1, R), f32)
    inp("ident", (128, 128), f32)
    if bi_nz[1]:
        inp("bi1h", (1, 3 * H), bf16)
        inp("bi1l", (1, 3 * H), bf16)
    if bhn_nz[1]:
        inp("bhn1h", (1, H), bf16)
        inp("bhn1l", (1, H), bf16)
    if bi_nz[2]:
        inp("bi2", (1, 3 * H), bf16)
    if bhn_nz[2]:
        inp("bhn2", (1, H), bf16)
    if ble_nz:
        inp("ble", (1, K), f32)

    out_h1 = nc.dram_tensor("h1f", [R, H], f32, kind="ExternalOutput")
    out_h2 = nc.dram_tensor("h2f", [R, H], f32, kind="ExternalOutput")
    out_lg = nc.dram_tensor("lg1", [R, TS * K], f32, kind="ExternalOutput")
    out_qv = nc.dram_tensor("qv2", [R, TS * A], f32, kind="ExternalOutput")

    with tile.TileContext(nc) as tc:
        _emit(nc, tc, din, out_h1, out_h2, out_lg, out_qv,
              bi_nz, bhn_nz, ble_nz, bemb1_nz, h0z, TS, f32, bf16, AF, ALU, AX)
    nc.compile()
    return nc


def _dedup_ldweights(nc):
    """Remove InstLdweights that reload the exact weights already resident:
    consecutive-on-the-PE-stream LDWs with an identical stationary AP (only
    non-transpose matmuls in between) are redundant -- the PE array still
    holds the data.  Waits from a removed LDW migrate to the next matmul."""
    from concourse import mybir

    removed = kept = 0
    for fn in nc.m.functions:
        for blk in fn.blocks:
            new_insts = []
            last_sig = None
            pending_waits = []
            for inst in blk.instructions:
                tn = type(inst).__name__
                if tn == "InstLdweights":
                    ap = inst.ins[-1]
                    sig = (ap.memref, ap.offset,
                           tuple(tuple(p) for p in ap.ap), str(ap.dtype),
                           inst.perf_mode, inst.is_transpose,
                           inst.tile_position)
                    if sig == last_sig:
                        removed += 1
                        si = inst.sync_info
                        if si is not None:
                            pending_waits.extend(si.on_wait)
                            assert not si.on_update, \
                                "removed LDW carries sem updates"
                        continue
                    last_sig = sig
                    kept += 1
                elif tn == "InstMatmult":
                    if pending_waits:
                        si = inst.sync_info
                        if si is None:
                            inst.sync_info = mybir.SyncInfo(
                                on_wait=list(pending_waits), on_update=[])
                        else:
                            have = {(w.id, w.wait_value) for w in si.on_wait}
                            for w in pending_waits:
                                if (w.id, w.wait_value) not in have:
                                    si.on_wait.append(w)
                        pending_waits = []
                new_insts.append(inst)
            assert not pending_waits, "dangling waits from removed LDW"
            blk.instructions[:] = new_insts
    print(f"[kernel] ldweights dedup: removed {removed}, kept {kept}")
    return nc


def _emit(nc, tc, din, out_h1, out_h2, out_lg, out_qv,
          bi_nz, bhn_nz, ble_nz, bemb1_nz, h0z, TS, f32, bf16, AF, ALU, AX):
    from contextlib import ExitStack

    ctx = ExitStack()
    with ctx:
        const = ctx.enter_context(tc.tile_pool(name="const", bufs=1))
        obs_p = ctx.enter_context(tc.tile_pool(name="obs", bufs=3))
        ae_p = ctx.enter_context(tc.tile_pool(name="aeT", bufs=4))
        st_p = ctx.enter_context(tc.tile_pool(name="state", bufs=2))
        tmp_p = ctx.enter_context(tc.tile_pool(name="tmp", bufs=8))
        outp = ctx.enter_context(tc.tile_pool(name="outs", bufs=1))
        ps_g = ctx.enter_context(tc.tile_pool(name="psg", bufs=5, space="PSUM"))
        ps_tr = ctx.enter_context(tc.tile_pool(name="pstr", bufs=2, space="PSUM"))
        ps_m = ctx.enter_context(tc.tile_pool(name="psm", bufs=1, space="PSUM"))

        def ctile(name, dt):
            t_ = const.tile(list(din[name].shape), dt, tag=name, name=name)
            nc.sync.dma_start(t_[:], din[name].ap())
            return t_

        # small embed-critical tiles first so the first obs/embed DMAs are
        # not queued behind ~10MB of GRU weights (PE can start ~40us earlier);
        # the big wi/wh DMAs are issued after the first embed groups.
        wi = {}
        wh = {}
        wemb = {1: {"h": ctile("wembedh", bf16), "l": ctile("wembedl", bf16)},
                2: {"h": ctile("wpol", bf16)}}
        # (data_sfx, weight_sfx) term lists
        terms = {1: (("h", "h"), ("l", "h"), ("h", "l")), 2: (("h", "h"),)}

        bemb = {1: ctile("bemb", f32), 2: ctile("bpol", f32)}
        ndt = {1: ctile("ndt1", f32), 2: ctile("ndt2", f32)}
        ndb_t = {}  # (gidx, br) -> (1, ng*R) sbuf tile, loaded with the group
        w1f = ctile("w1f", bf16)
        wle = ctile("wle", f32)
        b1f = ctile("b1f", bf16)
        onesb = ctile("onesb", bf16)
        onesf = ctile("onesf", f32)
        ident = ctile("ident", f32)
        bi = {}
        bhn = {}
        if bi_nz[1]:
            bi[1] = {"h": ctile("bi1h", bf16), "l": ctile("bi1l", bf16)}
        if bhn_nz[1]:
            bhn[1] = {"h": ctile("bhn1h", bf16), "l": ctile("bhn1l", bf16)}
        if bi_nz[2]:
            bi[2] = {"h": ctile("bi2", bf16)}
        if bhn_nz[2]:
            bhn[2] = {"h": ctile("bhn2", bf16)}
        ble = ctile("ble", f32) if ble_nz else None

        hB = {br: ctile(f"h{br}b0", f32) for br in (1, 2)}
        hT = {1: {"h": ctile("h1t0h", bf16), "l": ctile("h1t0l", bf16)},
              2: {"h": ctile("h2t0", bf16)}}

        # b1f broadcast across partitions, built once (rank-1 ones matmul)
        b1p = ps_m.tile([128, K * A], f32, tag="m", name="b1p")
        nc.tensor.matmul(b1p[:], onesb[:, :], b1f[:], start=True, stop=True)
        b1bc = outp.tile([128, K * A], f32, tag="b1bc")
        nc.vector.tensor_copy(b1bc[:], b1p[:])

        oh_sb = outp.tile([128, TS * K], f32, tag="oh")
        lg_sb = outp.tile([128, TS * K], f32, tag="lg")
        qv_sb = outp.tile([128, TS * A], f32, tag="qv")

        aeT = {1: {}, 2: {}}
        grp_w = {}

        aeT_groups = {}

        def emit_embed(gidx0, br):
            t0 = 4 * gidx0
            if t0 >= TS:
                return
            ng_t = min(4, TS - t0)  # timesteps in this obs group
            obs_d = {}
            for dsfx in ("h", "l") if br == 1 else ("h",):
                nm = f"obsT{br}{dsfx}" if br == 1 else "obsT2"
                ob = obs_p.tile([128, ng_t * R], bf16,
                                tag=f"ob{br}{dsfx}", name=f"ob{br}{dsfx}")
                nc.sync.dma_start(
                    ob[:].rearrange("d (t b) -> d t b", t=ng_t),
                    din[nm].ap()[t0:t0 + ng_t].rearrange("t d b -> d t b"))
                obs_d[dsfx] = ob
            nb = obs_p.tile([1, ng_t * R], bf16, tag=f"nb{br}", name=f"nb{br}",
                            bufs=3)
            nc.sync.dma_start(
                nb[:],
                din[f"ndb{br}"].ap().rearrange("(o t) b -> o (t b)", o=1)[
                    0:1, t0 * R:(t0 + ng_t) * R])
            ndBp4 = ps_tr.tile([128, ng_t * R], f32, tag="tr", name="ndBp4")
            nc.tensor.matmul(ndBp4[:], onesb[:, :], nb[:], start=True, stop=True)
            nd4 = st_p.tile([128, ng_t * R], f32, tag=f"nd4{br}", name=f"nd4{br}",
                            bufs=3)
            nc.vector.tensor_copy(nd4[:], ndBp4[:])
            ndb_t[(gidx0, br)] = nd4
            emb_terms = {1: (("h", "h"), ("l", "h"), ("h", "l")),
                         2: (("h", "h"),)}[br]
            if br == 1:
                ae1h = ae_p.tile([128, HC * ng_t * R], bf16, tag="ae1h",
                                 name="ae1h", bufs=3)
                ae1l = ae_p.tile([128, HC * ng_t * R], bf16, tag="ae1l",
                                 name="ae1l", bufs=3)
            else:
                ae2 = ae_p.tile([128, HC * ng_t * R], bf16, tag="ae2",
                                name="ae2", bufs=3)
            for hc in range(HC):
                pa = ps_m.tile([128, ng_t * R], f32, tag="m", name="pa")
                for i, (ds, ws) in enumerate(emb_terms):
                    nc.tensor.matmul(
                        pa[:], wemb[br][ws][:, hc * 128:(hc + 1) * 128],
                        obs_d[ds][:], start=(i == 0),
                        stop=(i == len(emb_terms) - 1), skip_group_check=True)
                sl = slice(hc * ng_t * R, (hc + 1) * ng_t * R)
                if br == 2:
                    nc.scalar.activation(ae2[:, sl], pa[:], AF.Relu,
                                         bias=bemb[2][:, hc:hc + 1])
                elif bemb1_nz:
                    aef = tmp_p.tile([128, ng_t * R], f32, tag="aef",
                                     bufs=1, name="aef")
                    nc.scalar.activation(aef[:], pa[:], AF.Relu,
                                         bias=bemb[1][:, hc:hc + 1])
                    nc.vector.tensor_copy(ae1h[:, sl], aef[:])
                    nc.vector.tensor_sub(ae1l[:, sl], aef[:], ae1h[:, sl])
                else:
                    nc.scalar.activation(ae1h[:, sl], pa[:], AF.Relu)
                    # ael = relu(ps) - aeh   (one fused DVE op)
                    nc.vector.scalar_tensor_tensor(
                        ae1l[:, sl], pa[:], 0.0, ae1h[:, sl],
                        ALU.max, ALU.subtract)
            if br == 1:
                aeT_groups[(gidx0, 1)] = ({"h": ae1h, "l": ae1l}, ng_t * R)
            else:
                aeT_groups[(gidx0, 2)] = ({"h": ae2}, ng_t * R)

        for brr in (1, 2):
            emit_embed(0, brr)
        wi[1] = {"h": ctile("wi1h", bf16), "l": ctile("wi1l", bf16)}
        wh[1] = {"h": ctile("wh1h", bf16), "l": ctile("wh1l", bf16)}
        wi[2] = {"h": ctile("wi2", bf16)}
        wh[2] = {"h": ctile("wh2", bf16)}
        for brr in (1, 2):
            emit_embed(1, brr)
        for t in range(TS):
            if t % 4 == 0:
                for brr in (1, 2):
                    aeT[brr], grp_w[brr] = aeT_groups.pop((t // 4, brr))
            tl = t % 4

            gidx = {"r": 0, "z": 1, "inn": 2, "hn": 2}

            def mm_phase(br):
                # gi + gh accumulated per gate bank; phases ordered so r/z
                # close early; kc-major, lhsT-grouped for ldweights dedup.
                g = {gate: ps_g.tile([128, 512], f32, tag="g", name=f"g{gate}")
                     for gate in ("r", "z", "hn", "inn")}
                dws = {}
                for ds, ws in terms[br]:
                    dws.setdefault(ds, []).append(ws)

                def gi_lhs(kc, ds):
                    return aeT[br][ds][:, kc * grp_w[br] + tl * R:
                                       kc * grp_w[br] + tl * R + R]

                def gh_lhs(kc, ds):
                    return hT[br][ds][:, kc * 128:(kc + 1) * 128]

                def wslice(w, ws, kc, gate):
                    return w[ws][:, kc * 1536 + gidx[gate] * 512:
                                 kc * 1536 + gidx[gate] * 512 + 512]

                skip_gh = h0z and t == 0
                sched = []  # (bank, lhsT_ap, rhs_ap)
                for kc in range(HC):
                    for ds, wss in dws.items():
                        for ws in wss:
                            for gate in ("r", "z"):
                                sched.append((gate, gi_lhs(kc, ds),
                                              wslice(wi[br], ws, kc, gate)))
                if bi_nz[br]:
                    for ws in bi[br]:
                        for gate in ("r", "z"):
                            sched.append((gate, onesb[:, :],
                                          bi[br][ws][:, gidx[gate] * 512:
                                                     gidx[gate] * 512 + 512]))
                if not skip_gh:
                    for kc in range(HC):
                        for ds, wss in dws.items():
                            for ws in wss:
                                for gate in ("r", "z"):
                                    sched.append((gate, gh_lhs(kc, ds),
                                                  wslice(wh[br], ws, kc, gate)))
                    for kc in range(HC):
                        for ds, wss in dws.items():
                            for ws in wss:
                                sched.append(("hn", gh_lhs(kc, ds),
                                              wslice(wh[br], ws, kc, "hn")))
                if bhn_nz[br]:
                    for ws in bhn[br]:
                        sched.append(("hn", onesb[:, :], bhn[br][ws][:, :]))
                for kc in range(HC):
                    for ds, wss in dws.items():
                        for ws in wss:
                            sched.append(("inn", gi_lhs(kc, ds),
                                          wslice(wi[br], ws, kc, "inn")))
                if bi_nz[br]:
                    for ws in bi[br]:
                        sched.append(("inn", onesb[:, :],
                                      bi[br][ws][:, gidx["inn"] * 512:
                                                 gidx["inn"] * 512 + 512]))
                total = {}
                for gate, _, _ in sched:
                    total[gate] = total.get(gate, 0) + 1
                seen = {gate: 0 for gate in total}
                for gate, lhs, rhs in sched:
                    seen[gate] += 1
                    nc.tensor.matmul(g[gate][:], lhs, rhs,
                                     start=(seen[gate] == 1),
                                     stop=(seen[gate] == total[gate]),
                                     skip_group_check=True)
                return g

            def gates_tail(br, g):
                skip_gh = h0z and t == 0
                zg = tmp_p.tile([128, 512], f32, tag="tmp", bufs=7, name="zg")
                nc.scalar.activation(zg[:], g["z"][:], AF.Sigmoid)
                ngt = tmp_p.tile([128, 512], f32, tag="tmp", bufs=7, name="ngt")
                if skip_gh:
                    nc.scalar.activation(ngt[:], g["inn"][:], AF.Tanh)
                else:
                    rg = tmp_p.tile([128, 512], f32, tag="tmp", bufs=7, name="rg")
                    nc.scalar.activation(rg[:], g["r"][:], AF.Sigmoid)
                    m = tmp_p.tile([128, 512], f32, tag="tmp", bufs=7, name="m")
                    nc.vector.tensor_tensor(m[:], rg[:], g["hn"][:], ALU.mult)
                    nin = tmp_p.tile([128, 512], f32, tag="tmp", bufs=7, name="nin")
                    nc.vector.tensor_tensor(nin[:], m[:], g["inn"][:], ALU.add)
                    nc.scalar.activation(ngt[:], nin[:], AF.Tanh)
                d = tmp_p.tile([128, 512], f32, tag="tmp", bufs=7, name="d")
                nc.vector.tensor_sub(d[:], hB[br][:], ngt[:])
                p_ = tmp_p.tile([128, 512], f32, tag="tmp", bufs=7, name="p_")
                nc.vector.tensor_mul(p_[:], zg[:], d[:])
                hnew = tmp_p.tile([128, 512], f32, tag="hnew", bufs=3, name="hnew")
                nc.vector.tensor_add(hnew[:], ngt[:], p_[:])
                if t == TS - 1:
                    nc.sync.dma_start((out_h1 if br == 1 else out_h2).ap(),
                                      hnew[:])
                # masked next-state (batch layout)
                hBn = st_p.tile([128, 512], f32, tag=f"hB{br}", name=f"hB{br}")
                nc.vector.tensor_scalar_mul(hBn[:], hnew[:], ndt[br][:, t:t + 1])
                hB[br] = hBn
                # single transpose set (unmasked); T-layout mask comes from a
                # rank-1 ones x nd-row matmul broadcast across partitions
                tru = ps_tr.tile([128, 512], f32, tag="tr", name="tru")
                for j in range(4):
                    nc.tensor.transpose(tru[:, j * 128:(j + 1) * 128],
                                        hnew[:, j * 128:(j + 1) * 128], ident[:])
                ndB3 = ndb_t[(t // 4, br)][:, tl * R:(tl + 1) * R].rearrange(
                    "p (o b) -> p o b", o=1).to_broadcast((128, 4, 128))
                tru3 = tru[:].rearrange("p (j b) -> p j b", j=4)
                if br == 1:
                    t1 = tmp_p.tile([128, 512], f32, tag="tmp", bufs=7, name="t1")
                    nc.vector.tensor_tensor(
                        t1[:].rearrange("p (j b) -> p j b", j=4), tru3, ndB3,
                        ALU.mult)
                    hTh = st_p.tile([128, 512], bf16, tag="hT1h", name="hT1h")
                    nc.scalar.activation(hTh[:], t1[:], AF.Copy)
                    hTl = st_p.tile([128, 512], bf16, tag="hT1l", name="hT1l")
                    nc.vector.tensor_sub(hTl[:], t1[:], hTh[:])
                    hT[1] = {"h": hTh, "l": hTl}
                else:
                    hT2n = st_p.tile([128, 512], bf16, tag="hT2", name="hT2")
                    nc.vector.tensor_tensor(
                        hT2n[:].rearrange("p (j b) -> p j b", j=4), tru3, ndB3,
                        ALU.mult)
                    hT[2] = {"h": hT2n}

                if br == 1:
                    # logits in true fp32 + one-hot
                    yT = st_p.tile([128, 512], f32, tag="yT1", name="yT1")
                    nc.scalar.activation(yT[:], tru[:], AF.Copy)
                    lgp = ps_m.tile([128, 512], f32, tag="m", name="lgp")
                    lmms = [(yT[:, kc * 128:(kc + 1) * 128],
                             wle[:, kc * K:(kc + 1) * K]) for kc in range(HC)]
                    if ble_nz:
                        lmms.append((onesf[:, :], ble[:, :]))
                    for i, (lhs, rhs) in enumerate(lmms):
                        nc.tensor.matmul(lgp[:, 0:K], lhs, rhs, start=(i == 0),
                                         stop=(i == len(lmms) - 1))
                    nc.vector.tensor_copy(lg_sb[:, t * K:(t + 1) * K], lgp[:, 0:K])
                    mx = tmp_p.tile([128, 1], f32, tag="mx", bufs=2, name="mx")
                    nc.vector.tensor_reduce(mx[:], lgp[:, 0:K], AX.X, ALU.max)
                    nc.vector.tensor_tensor(oh_sb[:, t * K:(t + 1) * K], lgp[:, 0:K],
                                            mx[:, 0:1].to_broadcast((128, K)),
                                            ALU.is_ge)
                else:
                    # q = y2 @ w1f + b1f ; q_vals = sum_k OH * q
                    yT = st_p.tile([128, 512], bf16, tag="yT2", name="yT2")
                    nc.scalar.activation(yT[:], tru[:], AF.Copy)
                    qp = ps_m.tile([128, 512], f32, tag="m", name="qp")
                    qmms = [(yT[:, kc * 128:(kc + 1) * 128],
                             w1f[:, kc * K * A:(kc + 1) * K * A]) for kc in range(HC)]
                    for i, (lhs, rhs) in enumerate(qmms):
                        nc.tensor.matmul(qp[:, 0:K * A], lhs, rhs, start=(i == 0),
                                         stop=(i == len(qmms) - 1))
                    qb = tmp_p.tile([128, K * A], f32, tag="qm", bufs=4, name="qb")
                    nc.vector.tensor_tensor(qb[:], qp[:, 0:K * A], b1bc[:], ALU.add)
                    qm = tmp_p.tile([128, K * A], f32, tag="qm", bufs=4, name="qm")
                    nc.vector.tensor_tensor(
                        qm[:].rearrange("p (a k) -> p a k", k=K),
                        qb[:].rearrange("p (a k) -> p a k", k=K),
                        oh_sb[:, t * K:(t + 1) * K].rearrange(
                            "p (o k) -> p o k", o=1).to_broadcast((128, A, K)),
                        ALU.mult)
                    nc.vector.tensor_reduce(qv_sb[:, t * A:(t + 1) * A],
                                            qm[:].rearrange("p (a k) -> p a k", k=K),
                                            AX.X, ALU.add)

            g1 = mm_phase(1)
            g2 = mm_phase(2)
            gates_tail(1, g1)
            if t % 4 == 0:
                emit_embed(t // 4 + 2, 1)
            elif t % 4 == 2:
                emit_embed(t // 4 + 2, 2)
            gates_tail(2, g2)

        nc.sync.dma_start(out_lg.ap(), lg_sb[:])
        nc.sync.dma_start(out_qv.ap(), qv_sb[:])


def _bf16_split(x):
    xh = x.astype(BF)
    xl = (x - xh.astype(np.float32)).astype(BF)
    return xh, xl


def _host_prep(inputs):
    f = lambda x: np.ascontiguousarray(np.asarray(x), dtype=np.float32)
    W_embed, b_embed = f(inputs["W_embed"]), f(inputs["b_embed"])
    Wi1, bi1 = f(inputs["Wi1"]), f(inputs["bi1"])
    Wh1, bhn1 = f(inputs["Wh1"]), f(inputs["bhn1"])
    W_sub, b_sub = f(inputs["W_sub"]), f(inputs["b_sub"])
    W_e1, b_e1 = f(inputs["W_e1"]), f(inputs["b_e1"])
    W_e2, b_e2 = f(inputs["W_e2"]), f(inputs["b_e2"])
    W_pol, b_pol = f(inputs["W_pol"]), f(inputs["b_pol"])
    Wi2, bi2 = f(inputs["Wi2"]), f(inputs["bi2"])
    Wh2, bhn2 = f(inputs["Wh2"]), f(inputs["bhn2"])
    W_w1, b_w1 = f(inputs["W_w1"]), f(inputs["b_w1"])
    W_b1, b_b1 = f(inputs["W_b1"]), f(inputs["b_b1"])

    e = np.tanh(np.maximum(W_e1 + b_e1, 0.0) @ W_e2 + b_e2)      # (K, S)
    W_le = W_sub @ e.T                                           # (H, K)
    b_le = b_sub @ e.T                                           # (K,)
    w1 = (e @ W_w1 + b_w1).reshape(K, H, A)                      # (K, H, A)
    b1 = e @ W_b1 + b_b1                                         # (K, A)
    # q matmul columns ordered (a, k): col a*K + k = w1[k, :, a]
    W1f = np.ascontiguousarray(w1.transpose(2, 0, 1).reshape(K * A, H).T)
    b1f = np.ascontiguousarray(b1.T.reshape(1, K * A))

    blk = lambda M: np.ascontiguousarray(
        np.concatenate([M[kc * 128:(kc + 1) * 128, :]
                        for kc in range(M.shape[0] // 128)], axis=1))
    wi1h, wi1l = _bf16_split(blk(Wi1))
    wh1h, wh1l = _bf16_split(blk(Wh1))
    wembedh, wembedl = _bf16_split(W_embed)
    prep = dict(
        wi1h=wi1h, wi1l=wi1l, wh1h=wh1h, wh1l=wh1l,
        wembedh=wembedh, wembedl=wembedl,
        wi2=blk(Wi2).astype(BF), wh2=blk(Wh2).astype(BF),
        wpol=W_pol.astype(BF),
        bemb=np.ascontiguousarray(b_embed.reshape(HC, 128).T),
        bpol=np.ascontiguousarray(b_pol.reshape(HC, 128).T),
        w1f=blk(W1f).astype(BF), wle=blk(W_le),
        b1f=b1f.astype(BF),
        onesb=np.ones((1, R), BF), onesf=np.ones((1, R), np.float32),
        ident=np.eye(128, dtype=np.float32),
        e=e,
    )
    flags = dict(
        bi1_nz=bool(np.any(bi1)), bhn1_nz=bool(np.any(bhn1)),
        bi2_nz=bool(np.any(bi2)), bhn2_nz=bool(np.any(bhn2)),
        ble_nz=bool(np.any(b_le)), bemb1_nz=bool(np.any(b_embed)),
        h0z=not (np.any(np.asarray(inputs["h1"])) or
                 np.any(np.asarray(inputs["h2"]))),
    )
    if flags["bi1_nz"]:
        prep["bi1h"], prep["bi1l"] = _bf16_split(bi1.reshape(1, 3 * H))
    if flags["bhn1_nz"]:
        prep["bhn1h"], prep["bhn1l"] = _bf16_split(bhn1.reshape(1, H))
    if flags["bi2_nz"]:
        prep["bi2"] = bi2.reshape(1, 3 * H).astype(BF)
    if flags["bhn2_nz"]:
        prep["bhn2"] = bhn2.reshape(1, H).astype(BF)
    if flags["ble_nz"]:
        prep["ble"] = b_le.reshape(1, K)
    return prep, flags


def _tlayout(h):
    """(R, H) batch-layout -> (128, H) T-layout (column block j = chunk j^T)."""
    return np.ascontiguousarray(
        h.T.reshape(HC, 128, R).transpose(1, 0, 2).reshape(128, HC * R))


def _core_inputs(inputs, prep, flags, c, t_steps):
    obs = np.asarray(inputs["obs"], dtype=np.float32)
    done = np.asarray(inputs["done"])
    nd = 1.0 - done.astype(np.float32)          # (T, NB)
    h1 = np.asarray(inputs["h1"], dtype=np.float32)
    h2 = np.asarray(inputs["h2"], dtype=np.float32)

    m = {}
    # branch 2: agent shard, rows nb-major
    rows2 = slice(c * R, (c + 1) * R)
    obs2 = np.ascontiguousarray(obs[:, c].transpose(0, 2, 1))[:t_steps]
    m["obsT2"] = obs2.astype(BF)
    nd2 = nd[:, rows2]                          # (T, R)
    ndn2 = np.vstack([nd2[1:], np.ones((1, R), np.float32)])
    m["ndt2"] = np.ascontiguousarray(ndn2.T)[:, :t_steps]
    m["ndb2"] = ndn2[:t_steps].astype(BF)
    h2m = h2[rows2] * nd2[0][:, None]
    m["h2b0"] = np.ascontiguousarray(h2m)
    m["h2t0"] = _tlayout(h2m).astype(BF)

    # branch 1: b-slice shard, rows b-major: local j = r_*8 + n, b = 16c + r_
    bsl = slice(16 * c, 16 * (c + 1))
    Tn = obs.shape[0]
    obs1 = obs[:, :, bsl, :].transpose(0, 2, 1, 3).reshape(Tn, R, D)
    obs1T = np.ascontiguousarray(obs1.transpose(0, 2, 1))[:t_steps]
    m["obsT1h"], m["obsT1l"] = _bf16_split(obs1T)
    done_v = done.reshape(Tn, N, B)
    nd1 = (1.0 - done_v[:, :, bsl].astype(np.float32)).transpose(0, 2, 1).reshape(Tn, R)
    ndn1 = np.vstack([nd1[1:], np.ones((1, R), np.float32)])
    m["ndt1"] = np.ascontiguousarray(ndn1.T)[:, :t_steps]
    m["ndb1"] = ndn1[:t_steps].astype(BF)
    h1_v = h1.reshape(N, B, H)[:, bsl].transpose(1, 0, 2).reshape(R, H)
    h1m = h1_v * nd1[0][:, None]
    m["h1b0"] = np.ascontiguousarray(h1m)
    m["h1t0h"], m["h1t0l"] = _bf16_split(_tlayout(h1m))

    for k in ("wi1h", "wi1l", "wh1h", "wh1l", "wembedh", "wembedl",
              "wi2", "wh2", "wpol", "bemb", "bpol", "w1f", "wle", "b1f",
              "onesb", "onesf", "ident"):
        m[k] = prep[k]
    for k, fl in (("bi1h", "bi1_nz"), ("bi1l", "bi1_nz"), ("bi2", "bi2_nz"),
                  ("bhn1h", "bhn1_nz"), ("bhn1l", "bhn1_nz"),
                  ("bhn2", "bhn2_nz"), ("ble", "ble_nz")):
        if flags[fl]:
            m[k] = prep[k]
    return m


def get_program(flags, t_steps=T):
    key = (flags["bi1_nz"], flags["bhn1_nz"], flags["bi2_nz"], flags["bhn2_nz"],
           flags["ble_nz"], flags["bemb1_nz"], flags["h0z"], t_steps)
    if key not in _CACHE:
        _CACHE[key] = _build_program(key)
    return _CACHE[key]


def assemble(results, prep, t_steps=T):
    """Gather per-core results into the full output pytree."""
    h1f = np.empty((NB, H), np.float32)
    h2f = np.empty((NB, H), np.float32)
    qv = np.empty((t_steps, NB, A), np.float32)
    lg = np.empty((t_steps, B, N, K), np.float32)
    for c in range(NCORES):
        r = results[c]
        h2f[c * R:(c + 1) * R] = r["h2f"]
        qv[:, c * R:(c + 1) * R] = r["qv2"].reshape(R, t_steps, A).transpose(1, 0, 2)
        # branch1 rows: j = r_*8 + n  -> (b = 16c + r_, n)
        l_ = r["lg1"].reshape(16, 8, t_steps, K)      # (r_, n, t, k)
        lg[:, 16 * c:16 * (c + 1)] = l_.transpose(2, 0, 1, 3)
        h1_ = r["h1f"].reshape(16, 8, H)              # (r_, n, H)
        h1f.reshape(N, B, H)[:, 16 * c:16 * (c + 1)] = h1_.transpose(1, 0, 2)
    se = np.broadcast_to(prep["e"], (t_steps, B, K, S)).copy()
    return ((h1f, h2f), qv, lg, se)


LAST_EXEC_NS = None


def kernel(_trace=False, **inputs):
    global LAST_EXEC_NS
    from concourse.bass_utils import run_bass_kernel_spmd

    prep, flags = _host_prep(inputs)
    nc = get_program(flags)
    in_maps = [_core_inputs(inputs, prep, flags, c, T) for c in range(NCORES)]
    core_ids = list(range(NCORES))
    if _trace:
        try:
            res = run_bass_kernel_spmd(nc, in_maps, core_ids=core_ids, trace=True)
        except Exception as e:  # trace infra (NTFF hook / upload) can fail
            print(f"[kernel] trace run failed ({e!r}); rerunning untraced")
            res = run_bass_kernel_spmd(nc, in_maps, core_ids=core_ids)
    else:
        res = run_bass_kernel_spmd(nc, in_maps, core_ids=core_ids)
    LAST_EXEC_NS = res.exec_time_ns
    return assemble(res.results, prep)


# revision 37
# speedup vs baseline: 1.2327x; 1.0023x over previous
"""Trainium2 Bass kernel for nn_AgentRNN: dual GRU scan + subtask/Q heads.

Sharding (8 cores, data-parallel, no collectives):
  - GRU2/Q path ("branch 2"): core c owns agent n=c -> rows [128c, 128(c+1))
    of the NB=1024 flat batch (nb-major).
  - GRU1/logits path ("branch 1"): core c owns batch columns b in
    [16c, 16(c+1)) across all 8 agents, with rows ordered b-major
    (local row j <-> (b = 16c + j//8, n = j%8)).
    With this choice, prob_flat row i == q row i live on the same core
    (prob_flat maps i -> (b=i//8, n=i%8), i.e. j == i - 128c), so the
    q_vals one-hot gather needs no cross-core traffic.

Precision:
  - Branch 1 feeds an argmax over logits whose top-2 gaps go down to ~3e-9,
    so its whole chain (embed, gi, gh) runs as 3-term bf16 split matmuls
    (x ~ xh+xl, W ~ Wh+Wl; terms xh@Wh + xl@Wh + xh@Wl), giving ~2^-17
    effective input precision at 3 bf16 matmul passes (vs 4 for true fp32).
    The logits head itself (y1 @ W_le) runs in true fp32.
  - Branch 2 only feeds Q *values* (no selection), so single-term bf16.

Device layout per core (R=128 rows per branch):
  - batch layout: partitions = local rows (128), free = H
  - T layout:     tiles (128, 512) where column block j holds
                  h[:, 128j:128(j+1)].T  (matmul lhsT operands)
"""
import sys
import os

sys.path.insert(0, "/opt/trn_rl_repo")

import numpy as np
import ml_dtypes

BF = ml_dtypes.bfloat16

T, N, B, D = 64, 8, 128, 128
H, S, K, A = 512, 64, 16, 16
NB = N * B
R = 128          # rows per core per branch
NCORES = 8
HC = H // 128    # 4 h-chunks

_CACHE = {}


def _build_program(cfg):
    """cfg: (bi1_nz, bhn1_nz, bi2_nz, bhn2_nz, ble_nz, bemb1_nz, t_steps)"""
    import concourse.bacc as bacc
    import concourse.tile as tile
    from concourse import mybir

    bi_nz = {1: cfg[0], 2: cfg[2]}
    bhn_nz = {1: cfg[1], 2: cfg[3]}
    ble_nz = cfg[4]
    bemb1_nz = cfg[5]
    h0z = cfg[6]
    TS = cfg[7]

    f32 = mybir.dt.float32
    bf16 = mybir.dt.bfloat16
    AF = mybir.ActivationFunctionType
    ALU = mybir.AluOpType
    AX = mybir.AxisListType

    nc = bacc.Bacc("TRN2", num_devices=NCORES)

    din = {}

    def inp(name, shape, dt):
        din[name] = nc.dram_tensor(name, list(shape), dt, kind="ExternalInput")
        return din[name]

    # branch 1 (split operands), branch 2 (single bf16)
    inp("obsT1h", (TS, D, R), bf16)
    inp("obsT1l", (TS, D, R), bf16)
    inp("obsT2", (TS, D, R), bf16)
    for sfx in ("h", "l"):
        inp(f"wi1{sfx}", (128, HC * 3 * H), bf16)
        inp(f"wh1{sfx}", (128, HC * 3 * H), bf16)
        inp(f"wembed{sfx}", (128, H), bf16)
    inp("wi2", (128, HC * 3 * H), bf16)
    inp("wh2", (128, HC * 3 * H), bf16)
    inp("wpol", (128, H), bf16)
    for br in (1, 2):
        inp(f"ndt{br}", (128, TS), f32)
        inp(f"ndb{br}", (TS, R), bf16)
        inp(f"h{br}b0", (R, H), f32)
    inp("h1t0h", (128, H), bf16)
    inp("h1t0l", (128, H), bf16)
    inp("h2t0", (128, H), bf16)
    inp("bemb", (128, HC), f32)
    inp("bpol", (128, HC), f32)
    inp("w1f", (128, HC * K * A), bf16)
    inp("wle", (128, HC * K), f32)
    inp("b1f", (1, K * A), bf16)
    inp("onesb", (1, R), bf16)
    inp("onesf", (1, R), f32)
    inp("ident", (128, 128), f32)
    if bi_nz[1]:
        inp("bi1h", (1, 3 * H), bf16)
        inp("bi1l", (1, 3 * H), bf16)
    if bhn_nz[1]:
        inp("bhn1h", (1, H), bf16)
        inp("bhn1l", (1, H), bf16)
    if bi_nz[2]:
        inp("bi2", (1, 3 * H), bf16)
    if bhn_nz[2]:
        inp("bhn2", (1, H), bf16)
    if ble_nz:
        inp("ble", (1, K), f32)

    out_h1 = nc.dram_tensor("h1f", [R, H], f32, kind="ExternalOutput")
    out_h2 = nc.dram_tensor("h2f", [R, H], f32, kind="ExternalOutput")
    out_lg = nc.dram_tensor("lg1", [R, TS * K], f32, kind="ExternalOutput")
    out_qv = nc.dram_tensor("qv2", [R, TS * A], f32, kind="ExternalOutput")

    with tile.TileContext(nc) as tc:
        _emit(nc, tc, din, out_h1, out_h2, out_lg, out_qv,
              bi_nz, bhn_nz, ble_nz, bemb1_nz, h0z, TS, f32, bf16, AF, ALU, AX)
    nc.compile()
    return nc


def _dedup_ldweights(nc):
    """Remove InstLdweights that reload the exact weights already resident:
    consecutive-on-the-PE-stream LDWs with an identical stationary AP (only
    non-transpose matmuls in between) are redundant -- the PE array still
    holds the data.  Waits from a removed LDW migrate to the next matmul."""
    from concourse import mybir

    removed = kept = 0
    for fn in nc.m.functions:
        for blk in fn.blocks:
            new_insts = []
            last_sig = None
            pending_waits = []
            for inst in blk.instructions:
                tn = type(inst).__name__
                if tn == "InstLdweights":
                    ap = inst.ins[-1]
                    sig = (ap.memref, ap.offset,
                           tuple(tuple(p) for p in ap.ap), str(ap.dtype),
                           inst.perf_mode, inst.is_transpose,
                           inst.tile_position)
                    if sig == last_sig:
                        removed += 1
                        si = inst.sync_info
                        if si is not None:
                            pending_waits.extend(si.on_wait)
                            assert not si.on_update, \
                                "removed LDW carries sem updates"
                        continue
                    last_sig = sig
                    kept += 1
                elif tn == "InstMatmult":
                    if pending_waits:
                        si = inst.sync_info
                        if si is None:
                            inst.sync_info = mybir.SyncInfo(
                                on_wait=list(pending_waits), on_update=[])
                        else:
                            have = {(w.id, w.wait_value) for w in si.on_wait}
                            for w in pending_waits:
                                if (w.id, w.wait_value) not in have:
                                    si.on_wait.append(w)
                        pending_waits = []
                new_insts.append(inst)
            assert not pending_waits, "dangling waits from removed LDW"
            blk.instructions[:] = new_insts
    print(f"[kernel] ldweights dedup: removed {removed}, kept {kept}")
    return nc


def _emit(nc, tc, din, out_h1, out_h2, out_lg, out_qv,
          bi_nz, bhn_nz, ble_nz, bemb1_nz, h0z, TS, f32, bf16, AF, ALU, AX):
    from contextlib import ExitStack

    ctx = ExitStack()
    with ctx:
        const = ctx.enter_context(tc.tile_pool(name="const", bufs=1))
        obs_p = ctx.enter_context(tc.tile_pool(name="obs", bufs=3))
        ae_p = ctx.enter_context(tc.tile_pool(name="aeT", bufs=4))
        st_p = ctx.enter_context(tc.tile_pool(name="state", bufs=2))
        tmp_p = ctx.enter_context(tc.tile_pool(name="tmp", bufs=8))
        outp = ctx.enter_context(tc.tile_pool(name="outs", bufs=1))
        ps_g = ctx.enter_context(tc.tile_pool(name="psg", bufs=6, space="PSUM"))
        ps_tr = ctx.enter_context(tc.tile_pool(name="pstr", bufs=1, space="PSUM"))
        ps_m = ctx.enter_context(tc.tile_pool(name="psm", bufs=1, space="PSUM"))

        def ctile(name, dt):
            t_ = const.tile(list(din[name].shape), dt, tag=name, name=name)
            nc.sync.dma_start(t_[:], din[name].ap())
            return t_

        # small embed-critical tiles first so the first obs/embed DMAs are
        # not queued behind ~10MB of GRU weights (PE can start ~40us earlier);
        # the big wi/wh DMAs are issued after the first embed groups.
        wi = {}
        wh = {}
        wemb = {1: {"h": ctile("wembedh", bf16), "l": ctile("wembedl", bf16)},
                2: {"h": ctile("wpol", bf16)}}
        # (data_sfx, weight_sfx) term lists
        terms = {1: (("h", "h"), ("l", "h"), ("h", "l")), 2: (("h", "h"),)}

        bemb = {1: ctile("bemb", f32), 2: ctile("bpol", f32)}
        ndt = {1: ctile("ndt1", f32), 2: ctile("ndt2", f32)}
        ndb_t = {}  # (gidx, br) -> (1, ng*R) sbuf tile, loaded with the group
        w1f = ctile("w1f", bf16)
        wle = ctile("wle", f32)
        b1f = ctile("b1f", bf16)
        onesb = ctile("onesb", bf16)
        onesf = ctile("onesf", f32)
        ident = ctile("ident", f32)
        bi = {}
        bhn = {}
        if bi_nz[1]:
            bi[1] = {"h": ctile("bi1h", bf16), "l": ctile("bi1l", bf16)}
        if bhn_nz[1]:
            bhn[1] = {"h": ctile("bhn1h", bf16), "l": ctile("bhn1l", bf16)}
        if bi_nz[2]:
            bi[2] = {"h": ctile("bi2", bf16)}
        if bhn_nz[2]:
            bhn[2] = {"h": ctile("bhn2", bf16)}
        ble = ctile("ble", f32) if ble_nz else None

        hB = {br: ctile(f"h{br}b0", f32) for br in (1, 2)}
        hT = {1: {"h": ctile("h1t0h", bf16), "l": ctile("h1t0l", bf16)},
              2: {"h": ctile("h2t0", bf16)}}

        # b1f broadcast across partitions, built once (rank-1 ones matmul)
        b1p = ps_m.tile([128, K * A], f32, tag="m", name="b1p")
        nc.tensor.matmul(b1p[:], onesb[:, :], b1f[:], start=True, stop=True)
        b1bc = outp.tile([128, K * A], f32, tag="b1bc")
        nc.vector.tensor_copy(b1bc[:], b1p[:])

        oh_sb = outp.tile([128, TS * K], f32, tag="oh")
        lg_sb = outp.tile([128, TS * K], f32, tag="lg")
        qv_sb = outp.tile([128, TS * A], f32, tag="qv")

        aeT = {1: {}, 2: {}}
        grp_w = {}

        aeT_groups = {}

        def emit_embed(gidx0, br):
            t0 = 4 * gidx0
            if t0 >= TS:
                return
            ng_t = min(4, TS - t0)  # timesteps in this obs group
            obs_d = {}
            for dsfx in ("h", "l") if br == 1 else ("h",):
                nm = f"obsT{br}{dsfx}" if br == 1 else "obsT2"
                ob = obs_p.tile([128, ng_t * R], bf16,
                                tag=f"ob{br}{dsfx}", name=f"ob{br}{dsfx}")
                nc.sync.dma_start(
                    ob[:].rearrange("d (t b) -> d t b", t=ng_t),
                    din[nm].ap()[t0:t0 + ng_t].rearrange("t d b -> d t b"))
                obs_d[dsfx] = ob
            nb = obs_p.tile([1, ng_t * R], bf16, tag=f"nb{br}", name=f"nb{br}",
                            bufs=3)
            nc.sync.dma_start(
                nb[:],
                din[f"ndb{br}"].ap().rearrange("(o t) b -> o (t b)", o=1)[
                    0:1, t0 * R:(t0 + ng_t) * R])
            ndBp4 = ps_tr.tile([128, ng_t * R], f32, tag="tr", name="ndBp4")
            nc.tensor.matmul(ndBp4[:], onesb[:, :], nb[:], start=True, stop=True)
            nd4 = st_p.tile([128, ng_t * R], f32, tag=f"nd4{br}", name=f"nd4{br}",
                            bufs=3)
            nc.vector.tensor_copy(nd4[:], ndBp4[:])
            ndb_t[(gidx0, br)] = nd4
            emb_terms = {1: (("h", "h"), ("l", "h"), ("h", "l")),
                         2: (("h", "h"),)}[br]
            if br == 1:
                ae1h = ae_p.tile([128, HC * ng_t * R], bf16, tag="ae1h",
                                 name="ae1h", bufs=3)
                ae1l = ae_p.tile([128, HC * ng_t * R], bf16, tag="ae1l",
                                 name="ae1l", bufs=3)
            else:
                ae2 = ae_p.tile([128, HC * ng_t * R], bf16, tag="ae2",
                                name="ae2", bufs=3)
            for hc in range(HC):
                pa = ps_m.tile([128, ng_t * R], f32, tag="m", name="pa")
                for i, (ds, ws) in enumerate(emb_terms):
                    nc.tensor.matmul(
                        pa[:], wemb[br][ws][:, hc * 128:(hc + 1) * 128],
                        obs_d[ds][:], start=(i == 0),
                        stop=(i == len(emb_terms) - 1), skip_group_check=True)
                sl = slice(hc * ng_t * R, (hc + 1) * ng_t * R)
                if br == 2:
                    nc.scalar.activation(ae2[:, sl], pa[:], AF.Relu,
                                         bias=bemb[2][:, hc:hc + 1])
                elif bemb1_nz:
                    aef = tmp_p.tile([128, ng_t * R], f32, tag="aef",
                                     bufs=1, name="aef")
                    nc.scalar.activation(aef[:], pa[:], AF.Relu,
                                         bias=bemb[1][:, hc:hc + 1])
                    nc.vector.tensor_copy(ae1h[:, sl], aef[:])
                    nc.vector.tensor_sub(ae1l[:, sl], aef[:], ae1h[:, sl])
                else:
                    nc.scalar.activation(ae1h[:, sl], pa[:], AF.Relu)
                    # ael = relu(ps) - aeh   (one fused DVE op)
                    nc.vector.scalar_tensor_tensor(
                        ae1l[:, sl], pa[:], 0.0, ae1h[:, sl],
                        ALU.max, ALU.subtract)
            if br == 1:
                aeT_groups[(gidx0, 1)] = ({"h": ae1h, "l": ae1l}, ng_t * R)
            else:
                aeT_groups[(gidx0, 2)] = ({"h": ae2}, ng_t * R)

        for brr in (1, 2):
            emit_embed(0, brr)
        wi[1] = {"h": ctile("wi1h", bf16), "l": ctile("wi1l", bf16)}
        wh[1] = {"h": ctile("wh1h", bf16), "l": ctile("wh1l", bf16)}
        wi[2] = {"h": ctile("wi2", bf16)}
        wh[2] = {"h": ctile("wh2", bf16)}
        for brr in (1, 2):
            emit_embed(1, brr)
        for t in range(TS):
            if t % 4 == 0:
                for brr in (1, 2):
                    aeT[brr], grp_w[brr] = aeT_groups.pop((t // 4, brr))
            tl = t % 4

            gidx = {"r": 0, "z": 1, "inn": 2, "hn": 2}

            def mm_phase(br):
                # gi + gh accumulated per gate bank; phases ordered so r/z
                # close early; kc-major, lhsT-grouped for ldweights dedup.
                g = {gate: ps_g.tile([128, 512], f32, tag="g", name=f"g{gate}")
                     for gate in ("r", "z", "hn", "inn")}
                dws = {}
                for ds, ws in terms[br]:
                    dws.setdefault(ds, []).append(ws)

                def gi_lhs(kc, ds):
                    return aeT[br][ds][:, kc * grp_w[br] + tl * R:
                                       kc * grp_w[br] + tl * R + R]

                def gh_lhs(kc, ds):
                    return hT[br][ds][:, kc * 128:(kc + 1) * 128]

                def wslice(w, ws, kc, gate):
                    return w[ws][:, kc * 1536 + gidx[gate] * 512:
                                 kc * 1536 + gidx[gate] * 512 + 512]

                skip_gh = h0z and t == 0
                sched = []  # (bank, lhsT_ap, rhs_ap)
                rz_gates = ("z",) if skip_gh else ("r", "z")
                for kc in range(HC):
                    for ds, wss in dws.items():
                        for ws in wss:
                            for gate in rz_gates:
                                sched.append((gate, gi_lhs(kc, ds),
                                              wslice(wi[br], ws, kc, gate)))
                if bi_nz[br]:
                    for ws in bi[br]:
                        for gate in rz_gates:
                            sched.append((gate, onesb[:, :],
                                          bi[br][ws][:, gidx[gate] * 512:
                                                     gidx[gate] * 512 + 512]))
                if not skip_gh:
                    for kc in range(HC):
                        for ds, wss in dws.items():
                            for ws in wss:
                                for gate in ("r", "z"):
                                    sched.append((gate, gh_lhs(kc, ds),
                                                  wslice(wh[br], ws, kc, gate)))
                    for kc in range(HC):
                        for ds, wss in dws.items():
                            for ws in wss:
                                sched.append(("hn", gh_lhs(kc, ds),
                                              wslice(wh[br], ws, kc, "hn")))
                if bhn_nz[br]:
                    for ws in bhn[br]:
                        sched.append(("hn", onesb[:, :], bhn[br][ws][:, :]))
                for kc in range(HC):
                    for ds, wss in dws.items():
                        for ws in wss:
                            sched.append(("inn", gi_lhs(kc, ds),
                                          wslice(wi[br], ws, kc, "inn")))
                if bi_nz[br]:
                    for ws in bi[br]:
                        sched.append(("inn", onesb[:, :],
                                      bi[br][ws][:, gidx["inn"] * 512:
                                                 gidx["inn"] * 512 + 512]))
                total = {}
                for gate, _, _ in sched:
                    total[gate] = total.get(gate, 0) + 1
                seen = {gate: 0 for gate in total}
                for gate, lhs, rhs in sched:
                    seen[gate] += 1
                    nc.tensor.matmul(g[gate][:], lhs, rhs,
                                     start=(seen[gate] == 1),
                                     stop=(seen[gate] == total[gate]),
                                     skip_group_check=True)
                return g

            def gates_tail(br, g):
                skip_gh = h0z and t == 0
                zg = tmp_p.tile([128, 512], f32, tag="tmp", bufs=7, name="zg")
                nc.scalar.activation(zg[:], g["z"][:], AF.Sigmoid)
                ngt = tmp_p.tile([128, 512], f32, tag="tmp", bufs=7, name="ngt")
                if skip_gh:
                    nc.scalar.activation(ngt[:], g["inn"][:], AF.Tanh)
                else:
                    rg = tmp_p.tile([128, 512], f32, tag="tmp", bufs=7, name="rg")
                    nc.scalar.activation(rg[:], g["r"][:], AF.Sigmoid)
                    m = tmp_p.tile([128, 512], f32, tag="tmp", bufs=7, name="m")
                    nc.vector.tensor_tensor(m[:], rg[:], g["hn"][:], ALU.mult)
                    nin = tmp_p.tile([128, 512], f32, tag="tmp", bufs=7, name="nin")
                    nc.vector.tensor_tensor(nin[:], m[:], g["inn"][:], ALU.add)
                    nc.scalar.activation(ngt[:], nin[:], AF.Tanh)
                d = tmp_p.tile([128, 512], f32, tag="tmp", bufs=7, name="d")
                nc.vector.tensor_sub(d[:], hB[br][:], ngt[:])
                p_ = tmp_p.tile([128, 512], f32, tag="tmp", bufs=7, name="p_")
                nc.vector.tensor_mul(p_[:], zg[:], d[:])
                hnew = tmp_p.tile([128, 512], f32, tag="hnew", bufs=3, name="hnew")
                nc.vector.tensor_add(hnew[:], ngt[:], p_[:])
                if t == TS - 1:
                    nc.sync.dma_start((out_h1 if br == 1 else out_h2).ap(),
                                      hnew[:])
                # masked next-state (batch layout)
                hBn = st_p.tile([128, 512], f32, tag=f"hB{br}", name=f"hB{br}")
                nc.vector.tensor_scalar_mul(hBn[:], hnew[:], ndt[br][:, t:t + 1])
                hB[br] = hBn
                # single transpose set (unmasked); T-layout mask comes from a
                # rank-1 ones x nd-row matmul broadcast across partitions
                tru = ps_tr.tile([128, 512], f32, tag="tr", name="tru")
                for j in range(4):
                    nc.tensor.transpose(tru[:, j * 128:(j + 1) * 128],
                                        hnew[:, j * 128:(j + 1) * 128], ident[:])
                ndB3 = ndb_t[(t // 4, br)][:, tl * R:(tl + 1) * R].rearrange(
                    "p (o b) -> p o b", o=1).to_broadcast((128, 4, 128))
                tru3 = tru[:].rearrange("p (j b) -> p j b", j=4)
                if br == 1:
                    t1 = tmp_p.tile([128, 512], f32, tag="tmp", bufs=7, name="t1")
                    nc.vector.tensor_tensor(
                        t1[:].rearrange("p (j b) -> p j b", j=4), tru3, ndB3,
                        ALU.mult)
                    hTh = st_p.tile([128, 512], bf16, tag="hT1h", name="hT1h")
                    nc.scalar.activation(hTh[:], t1[:], AF.Copy)
                    hTl = st_p.tile([128, 512], bf16, tag="hT1l", name="hT1l")
                    nc.vector.tensor_sub(hTl[:], t1[:], hTh[:])
                    hT[1] = {"h": hTh, "l": hTl}
                else:
                    hT2n = st_p.tile([128, 512], bf16, tag="hT2", name="hT2")
                    nc.vector.tensor_tensor(
                        hT2n[:].rearrange("p (j b) -> p j b", j=4), tru3, ndB3,
                        ALU.mult)
                    hT[2] = {"h": hT2n}

                if br == 1:
                    # logits in true fp32 + one-hot
                    yT = st_p.tile([128, 512], f32, tag="yT1", name="yT1")
                    nc.scalar.activation(yT[:], tru[:], AF.Copy)
                    lgp = ps_m.tile([128, 512], f32, tag="m", name="lgp")
                    lmms = [(yT[:, kc * 128:(kc + 1) * 128],
                             wle[:, kc * K:(kc + 1) * K]) for kc in range(HC)]
                    if ble_nz:
                        lmms.append((onesf[:, :], ble[:, :]))
                    for i, (lhs, rhs) in enumerate(lmms):
                        nc.tensor.matmul(lgp[:, 0:K], lhs, rhs, start=(i == 0),
                                         stop=(i == len(lmms) - 1))
                    nc.vector.tensor_copy(lg_sb[:, t * K:(t + 1) * K], lgp[:, 0:K])
                    mx = tmp_p.tile([128, 1], f32, tag="mx", bufs=2, name="mx")
                    nc.vector.tensor_reduce(mx[:], lgp[:, 0:K], AX.X, ALU.max)
                    nc.vector.tensor_tensor(oh_sb[:, t * K:(t + 1) * K], lgp[:, 0:K],
                                            mx[:, 0:1].to_broadcast((128, K)),
                                            ALU.is_ge)
                else:
                    # q = y2 @ w1f + b1f ; q_vals = sum_k OH * q
                    yT = st_p.tile([128, 512], bf16, tag="yT2", name="yT2")
                    nc.scalar.activation(yT[:], tru[:], AF.Copy)
                    qp = ps_m.tile([128, 512], f32, tag="m", name="qp")
                    qmms = [(yT[:, kc * 128:(kc + 1) * 128],
                             w1f[:, kc * K * A:(kc + 1) * K * A]) for kc in range(HC)]
                    for i, (lhs, rhs) in enumerate(qmms):
                        nc.tensor.matmul(qp[:, 0:K * A], lhs, rhs, start=(i == 0),
                                         stop=(i == len(qmms) - 1))
                    qb = tmp_p.tile([128, K * A], f32, tag="qm", bufs=4, name="qb")
                    nc.vector.tensor_tensor(qb[:], qp[:, 0:K * A], b1bc[:], ALU.add)
                    qm = tmp_p.tile([128, K * A], f32, tag="qm", bufs=4, name="qm")
                    nc.vector.tensor_tensor(
                        qm[:].rearrange("p (a k) -> p a k", k=K),
                        qb[:].rearrange("p (a k) -> p a k", k=K),
                        oh_sb[:, t * K:(t + 1) * K].rearrange(
                            "p (o k) -> p o k", o=1).to_broadcast((128, A, K)),
                        ALU.mult)
                    nc.vector.tensor_reduce(qv_sb[:, t * A:(t + 1) * A],
                                            qm[:].rearrange("p (a k) -> p a k", k=K),
                                            AX.X, ALU.add)

            g1 = mm_phase(1)
            g2 = mm_phase(2)
            gates_tail(1, g1)
            if t % 4 == 0:
                emit_embed(t // 4 + 2, 1)
            elif t % 4 == 2:
                emit_embed(t // 4 + 2, 2)
            gates_tail(2, g2)

        nc.sync.dma_start(out_lg.ap(), lg_sb[:])
        nc.sync.dma_start(out_qv.ap(), qv_sb[:])


def _bf16_split(x):
    xh = x.astype(BF)
    xl = (x - xh.astype(np.float32)).astype(BF)
    return xh, xl


def _host_prep(inputs):
    f = lambda x: np.ascontiguousarray(np.asarray(x), dtype=np.float32)
    W_embed, b_embed = f(inputs["W_embed"]), f(inputs["b_embed"])
    Wi1, bi1 = f(inputs["Wi1"]), f(inputs["bi1"])
    Wh1, bhn1 = f(inputs["Wh1"]), f(inputs["bhn1"])
    W_sub, b_sub = f(inputs["W_sub"]), f(inputs["b_sub"])
    W_e1, b_e1 = f(inputs["W_e1"]), f(inputs["b_e1"])
    W_e2, b_e2 = f(inputs["W_e2"]), f(inputs["b_e2"])
    W_pol, b_pol = f(inputs["W_pol"]), f(inputs["b_pol"])
    Wi2, bi2 = f(inputs["Wi2"]), f(inputs["bi2"])
    Wh2, bhn2 = f(inputs["Wh2"]), f(inputs["bhn2"])
    W_w1, b_w1 = f(inputs["W_w1"]), f(inputs["b_w1"])
    W_b1, b_b1 = f(inputs["W_b1"]), f(inputs["b_b1"])

    e = np.tanh(np.maximum(W_e1 + b_e1, 0.0) @ W_e2 + b_e2)      # (K, S)
    W_le = W_sub @ e.T                                           # (H, K)
    b_le = b_sub @ e.T                                           # (K,)
    w1 = (e @ W_w1 + b_w1).reshape(K, H, A)                      # (K, H, A)
    b1 = e @ W_b1 + b_b1                                         # (K, A)
    # q matmul columns ordered (a, k): col a*K + k = w1[k, :, a]
    W1f = np.ascontiguousarray(w1.transpose(2, 0, 1).reshape(K * A, H).T)
    b1f = np.ascontiguousarray(b1.T.reshape(1, K * A))

    blk = lambda M: np.ascontiguousarray(
        np.concatenate([M[kc * 128:(kc + 1) * 128, :]
                        for kc in range(M.shape[0] // 128)], axis=1))
    wi1h, wi1l = _bf16_split(blk(Wi1))
    wh1h, wh1l = _bf16_split(blk(Wh1))
    wembedh, wembedl = _bf16_split(W_embed)
    prep = dict(
        wi1h=wi1h, wi1l=wi1l, wh1h=wh1h, wh1l=wh1l,
        wembedh=wembedh, wembedl=wembedl,
        wi2=blk(Wi2).astype(BF), wh2=blk(Wh2).astype(BF),
        wpol=W_pol.astype(BF),
        bemb=np.ascontiguousarray(b_embed.reshape(HC, 128).T),
        bpol=np.ascontiguousarray(b_pol.reshape(HC, 128).T),
        w1f=blk(W1f).astype(BF), wle=blk(W_le),
        b1f=b1f.astype(BF),
        onesb=np.ones((1, R), BF), onesf=np.ones((1, R), np.float32),
        ident=np.eye(128, dtype=np.float32),
        e=e,
    )
    flags = dict(
        bi1_nz=bool(np.any(bi1)), bhn1_nz=bool(np.any(bhn1)),
        bi2_nz=bool(np.any(bi2)), bhn2_nz=bool(np.any(bhn2)),
        ble_nz=bool(np.any(b_le)), bemb1_nz=bool(np.any(b_embed)),
        h0z=not (np.any(np.asarray(inputs["h1"])) or
                 np.any(np.asarray(inputs["h2"]))),
    )
    if flags["bi1_nz"]:
        prep["bi1h"], prep["bi1l"] = _bf16_split(bi1.reshape(1, 3 * H))
    if flags["bhn1_nz"]:
        prep["bhn1h"], prep["bhn1l"] = _bf16_split(bhn1.reshape(1, H))
    if flags["bi2_nz"]:
        prep["bi2"] = bi2.reshape(1, 3 * H).astype(BF)
    if flags["bhn2_nz"]:
        prep["bhn2"] = bhn2.reshape(1, H).astype(BF)
    if flags["ble_nz"]:
        prep["ble"] = b_le.reshape(1, K)
    return prep, flags


def _tlayout(h):
    """(R, H) batch-layout -> (128, H) T-layout (column block j = chunk j^T)."""
    return np.ascontiguousarray(
        h.T.reshape(HC, 128, R).transpose(1, 0, 2).reshape(128, HC * R))


def _core_inputs(inputs, prep, flags, c, t_steps):
    obs = np.asarray(inputs["obs"], dtype=np.float32)
    done = np.asarray(inputs["done"])
    nd = 1.0 - done.astype(np.float32)          # (T, NB)
    h1 = np.asarray(inputs["h1"], dtype=np.float32)
    h2 = np.asarray(inputs["h2"], dtype=np.float32)

    m = {}
    # branch 2: agent shard, rows nb-major
    rows2 = slice(c * R, (c + 1) * R)
    obs2 = np.ascontiguousarray(obs[:, c].transpose(0, 2, 1))[:t_steps]
    m["obsT2"] = obs2.astype(BF)
    nd2 = nd[:, rows2]                          # (T, R)
    ndn2 = np.vstack([nd2[1:], np.ones((1, R), np.float32)])
    m["ndt2"] = np.ascontiguousarray(ndn2.T)[:, :t_steps]
    m["ndb2"] = ndn2[:t_steps].astype(BF)
    h2m = h2[rows2] * nd2[0][:, None]
    m["h2b0"] = np.ascontiguousarray(h2m)
    m["h2t0"] = _tlayout(h2m).astype(BF)

    # branch 1: b-slice shard, rows b-major: local j = r_*8 + n, b = 16c + r_
    bsl = slice(16 * c, 16 * (c + 1))
    Tn = obs.shape[0]
    obs1 = obs[:, :, bsl, :].transpose(0, 2, 1, 3).reshape(Tn, R, D)
    obs1T = np.ascontiguousarray(obs1.transpose(0, 2, 1))[:t_steps]
    m["obsT1h"], m["obsT1l"] = _bf16_split(obs1T)
    done_v = done.reshape(Tn, N, B)
    nd1 = (1.0 - done_v[:, :, bsl].astype(np.float32)).transpose(0, 2, 1).reshape(Tn, R)
    ndn1 = np.vstack([nd1[1:], np.ones((1, R), np.float32)])
    m["ndt1"] = np.ascontiguousarray(ndn1.T)[:, :t_steps]
    m["ndb1"] = ndn1[:t_steps].astype(BF)
    h1_v = h1.reshape(N, B, H)[:, bsl].transpose(1, 0, 2).reshape(R, H)
    h1m = h1_v * nd1[0][:, None]
    m["h1b0"] = np.ascontiguousarray(h1m)
    m["h1t0h"], m["h1t0l"] = _bf16_split(_tlayout(h1m))

    for k in ("wi1h", "wi1l", "wh1h", "wh1l", "wembedh", "wembedl",
              "wi2", "wh2", "wpol", "bemb", "bpol", "w1f", "wle", "b1f",
              "onesb", "onesf", "ident"):
        m[k] = prep[k]
    for k, fl in (("bi1h", "bi1_nz"), ("bi1l", "bi1_nz"), ("bi2", "bi2_nz"),
                  ("bhn1h", "bhn1_nz"), ("bhn1l", "bhn1_nz"),
                  ("bhn2", "bhn2_nz"), ("ble", "ble_nz")):
        if flags[fl]:
            m[k] = prep[k]
    return m


def get_program(flags, t_steps=T):
    key = (flags["bi1_nz"], flags["bhn1_nz"], flags["bi2_nz"], flags["bhn2_nz"],
           flags["ble_nz"], flags["bemb1_nz"], flags["h0z"], t_steps)
    if key not in _CACHE:
        _CACHE[key] = _build_program(key)
    return _CACHE[key]


def assemble(results, prep, t_steps=T):
    """Gather per-core results into the full output pytree."""
    h1f = np.empty((NB, H), np.float32)
    h2f = np.empty((NB, H), np.float32)
    qv = np.empty((t_steps, NB, A), np.float32)
    lg = np.empty((t_steps, B, N, K), np.float32)
    for c in range(NCORES):
        r = results[c]
        h2f[c * R:(c + 1) * R] = r["h2f"]
        qv[:, c * R:(c + 1) * R] = r["qv2"].reshape(R, t_steps, A).transpose(1, 0, 2)
        # branch1 rows: j = r_*8 + n  -> (b = 16c + r_, n)
        l_ = r["lg1"].reshape(16, 8, t_steps, K)      # (r_, n, t, k)
        lg[:, 16 * c:16 * (c + 1)] = l_.transpose(2, 0, 1, 3)
        h1_ = r["h1f"].reshape(16, 8, H)              # (r_, n, H)
        h1f.reshape(N, B, H)[:, 16 * c:16 * (c + 1)] = h1_.transpose(1, 0, 2)
    se = np.broadcast_to(prep["e"], (t_steps, B, K, S)).copy()
    return ((h1f, h2f), qv, lg, se)


LAST_EXEC_NS = None


def kernel(_trace=False, **inputs):
    global LAST_EXEC_NS
    from concourse.bass_utils import run_bass_kernel_spmd

    prep, flags = _host_prep(inputs)
    nc = get_program(flags)
    in_maps = [_core_inputs(inputs, prep, flags, c, T) for c in range(NCORES)]
    core_ids = list(range(NCORES))
    if _trace:
        try:
            res = run_bass_kernel_spmd(nc, in_maps, core_ids=core_ids, trace=True)
        except Exception as e:  # trace infra (NTFF hook / upload) can fail
            print(f"[kernel] trace run failed ({e!r}); rerunning untraced")
            res = run_bass_kernel_spmd(nc, in_maps, core_ids=core_ids)
    else:
        res = run_bass_kernel_spmd(nc, in_maps, core_ids=core_ids)
    LAST_EXEC_NS = res.exec_time_ns
    return assemble(res.results, prep)
